# revision 42
# baseline (speedup 1.0000x reference)
"""Trainium2 Bass kernel for nn_AdjacencyConv (GNN message passing).

Reference computation:
    msg  = relu(concat[x[src], x_bridge[bri]] @ lin_w.T + lin_b)   # [E, D]
    agg  = segment_sum(msg, dst, N)                                # [N, D]
    out  = agg + (1+eps)*x
    h    = relu(BN(out @ w1.T + b1)); h = relu(BN(h @ w2.T + b2))  # train-mode BN

Device algorithm (8-core SPMD, edges sharded by dst node-tile):
  Phase A (per core, replicated): build bf16 node tables in DRAM
        xW = x @ Wx.T            (Wx = lin_w[:, :D])      rows [0, N2)
        eW = x_bridge @ Wb.T + b (Wb = lin_w[:, D:])      rows [N2, N2+Nb2)
    so the per-edge linear factorizes: msg = relu(xW[src] + eW[bri]).
  Phase B: per dst node-tile of 128 nodes, one combined dma_gather per
    chunk fetches both rows of each edge from the concatenated table
    (edge-major bf16 [128e, D]), add + relu, then scatter-add via one-hot
    matmuls accumulating feature-major agg fp32 in PSUM. Residual and the
    first MLP linear run per-tile inside the loop. Phase B is bound by the
    SWDGE drain rate (~2.5 ns/descriptor across the 4 queues); bf16 halves
    gather bytes and one-pass matmuls vs fp32.
  Tail: 2 small follow-up NEFFs apply train-mode BN (biases b1/b2 cancel
    in BN and are dropped); the [128, 2] BN stat reduction across cores is
    host-stitched between NEFFs — a device AllReduce measured ~200us worse
    because it absorbs cross-core launch skew into the NEFF exec time.

Host side does only layout preprocessing: index sorting/padding/packing,
weight transposes, the 2KB BN stat reductions, and output assembly.
"""

import os
import numpy as np
import ml_dtypes

BF16 = ml_dtypes.bfloat16
N, NB, E, D = 10000, 20000, 640000, 128
P = 128
NCORES = 8
NTC = 10                    # node tiles per core
N2 = NCORES * NTC * P       # 10240 padded nodes
NB2 = 20096                 # x_bridge padded to multiple of 128
BN_EPS = 1e-5
GCHUNK = int(os.environ.get("KGCHUNK", "8"))  # batches per dma_gather
NQ = int(os.environ.get("KNQ", "4"))   # SWDGE queues
GBUFS = int(os.environ.get("KGBUFS", "10"))  # gather tile-pool depth
PRIME = int(os.environ.get("KPRIME", "20"))  # split-gather prefix chunks

_cache = {}

# exposed for test.py
last_results = None


def _chunks(B):
    """Chunk enumeration shared by host prep and kernel build:
    (lt, done, cb, batch_off, first_of_tile, last_of_tile)."""
    out = []
    gbase = 0
    for lt in range(NTC):
        nb = int(B[lt])
        done = 0
        while done < nb:
            cb = min(GCHUNK, nb - done)
            out.append((lt, done, cb, gbase + done,
                        done == 0, done + cb == nb))
            done += cb
        gbase += nb
    return out


def _pack_idx(idx16):
    """Wrap an int16 index vector for dma_gather: [16, n/16] replicated x8."""
    w = idx16.reshape(-1, 16).T
    return np.tile(w, (8, 1)).copy()


def _host_prep(inputs):
    x = np.asarray(inputs["x"], np.float32)
    xb = np.asarray(inputs["x_bridge"], np.float32)
    ei = np.asarray(inputs["edge_index"])
    bri = np.asarray(inputs["bridge_index"])
    lin_w = np.asarray(inputs["lin_w"], np.float32)
    lin_b = np.asarray(inputs["lin_b"], np.float32)
    eps = float(np.asarray(inputs["eps"]).reshape(-1)[0])
    w1 = np.asarray(inputs["w1"], np.float32)
    g1 = np.asarray(inputs["g1"], np.float32)
    beta1 = np.asarray(inputs["beta1"], np.float32)
    w2 = np.asarray(inputs["w2"], np.float32)
    g2 = np.asarray(inputs["g2"], np.float32)
    beta2 = np.asarray(inputs["beta2"], np.float32)

    src = ei[0].astype(np.int64)
    dst = ei[1].astype(np.int64)
    bri = bri.astype(np.int64)

    # sort edges by dst, bucket into 128-node tiles
    order = np.argsort(dst, kind="stable")
    dsts = dst[order]
    srcs = src[order]
    bris = bri[order]
    gt_bounds = np.searchsorted(dsts, np.arange(NCORES * NTC + 1) * P)

    # uniform program structure: per local tile lt, same batch count across cores
    cnt = np.diff(gt_bounds)  # edges per global tile (len 80)
    cnt = cnt.reshape(NCORES, NTC)
    ceils = -(-cnt // P)
    B = np.maximum(1, ceils.max(axis=0))  # batches per local tile (len NTC)
    SB = int(B.sum())          # total batches per core
    S = SB * P                 # padded edges per core

    src_pad = np.zeros((NCORES, S), np.int64)
    bri_pad = np.zeros((NCORES, S), np.int64)
    dloc_pad = np.full((NCORES, S), 999.0, np.float32)
    for c in range(NCORES):
        off = 0
        for lt in range(NTC):
            gt = c * NTC + lt
            a, b = gt_bounds[gt], gt_bounds[gt + 1]
            n = b - a
            src_pad[c, off:off + n] = srcs[a:b]
            bri_pad[c, off:off + n] = bris[a:b]
            dloc_pad[c, off:off + n] = (dsts[a:b] - gt * P).astype(np.float32)
            off += B[lt] * P
    # dstloc transposed: [128, SB], column j = batch j's 128 local-dst values
    dloc_T = np.ascontiguousarray(
        dloc_pad.reshape(NCORES, SB, P).transpose(0, 2, 1)).astype(BF16)

    # Gather index streams. The first `prime` chunks use split x/e gathers
    # (x-gathers only depend on the xW half-table, so they start while eW
    # still builds); the rest use one combined gather per chunk:
    # [src idx (cb*128), bri idx + N2 (cb*128)].
    chunks = _chunks(B)
    prime = min(PRIME, len(chunks))
    ep = sum(ch[2] for ch in chunks[:prime]) * P  # prefix edges
    comb = np.zeros((NCORES, 2 * (S - ep)), np.int64)
    for c in range(NCORES):
        woff = 0
        for (lt, done, cb, boff, _, _) in chunks[prime:]:
            off = boff * P
            ne = cb * P
            comb[c, woff:woff + ne] = src_pad[c, off:off + ne]
            comb[c, woff + ne:woff + 2 * ne] = bri_pad[c, off:off + ne] + N2
            woff += 2 * ne



    # feature-major padded inputs for the table build (bf16)
    xt = np.zeros((D, N2), BF16)
    xt[:, :N] = x.T.astype(BF16)
    xbt = np.zeros((D, NB2), BF16)
    xbt[:, :NB] = xb.T.astype(BF16)

    wxt = np.ascontiguousarray(lin_w[:, :D].T).astype(BF16)   # [in_f, out]
    wbt = np.ascontiguousarray(lin_w[:, D:].T).astype(BF16)
    w1t = np.ascontiguousarray(w1.T)
    w2t = np.ascontiguousarray(w2.T)
    linbb = np.tile(lin_b[None, :], (P, 1)).astype(np.float32)   # [128, 128]
    iotab = np.tile(np.arange(P, dtype=np.float32)[None, :], (P, GCHUNK)).astype(BF16)
    ident = np.eye(P, dtype=np.float32)

    g1c = np.ascontiguousarray(g1[:, None])
    b1c = np.ascontiguousarray(beta1[:, None])
    g2c = np.ascontiguousarray(g2[:, None])
    b2c = np.ascontiguousarray(beta2[:, None])

    # per-core residual slice (feature-major) and validity mask
    span = NTC * P
    xct = np.zeros((NCORES, D, span), np.float32)
    maskb = np.zeros((NCORES, P, span), np.float32)
    for c in range(NCORES):
        c0 = c * span
        v = min(max(N - c0, 0), span)
        if v > 0:
            xct[c, :, :v] = x.T[:, c0:c0 + v]
            maskb[c, :, :v] = 1.0

    in_maps = []
    for c in range(NCORES):
        in_maps.append({
            "xt": xt, "xbt": xbt,
            "wxt": wxt, "wbt": wbt, "w1t": w1t, "w2t": w2t,
            "linbb": linbb, "iotab": iotab, "ident": ident,
            "g1c": g1c, "b1c": b1c, "g2c": g2c, "b2c": b2c,
            "xct": np.ascontiguousarray(xct[c]),
            "maskb": np.ascontiguousarray(maskb[c]),
            "combw": _pack_idx(comb[c].astype(np.int16)),
            "srcwp": _pack_idx(src_pad[c, :ep].astype(np.int16)),
            "briwp": _pack_idx(bri_pad[c, :ep].astype(np.int16)),
            "dstloc": np.ascontiguousarray(dloc_T[c]),
        })
    meta = (tuple(int(b) for b in B), 1.0 + eps)
    return in_maps, meta


def _build(meta):
    import concourse.bacc as bacc
    import concourse.mybir as mybir
    import concourse.tile as tile

    KP = 3  # full fused kernel
    B, resid_scale = meta
    SB = sum(B)
    S = SB * P
    f32 = mybir.dt.float32
    bf16 = mybir.dt.bfloat16
    i16 = mybir.dt.int16
    Alu = mybir.AluOpType
    Act = mybir.ActivationFunctionType
    span = NTC * P

    NQr = int(os.environ.get("KNQ", "4"))
    scratch = int(os.environ.get("KSCRATCH", "16384"))
    nc = bacc.Bacc("TRN2", target_bir_lowering=False, debug=False,
                   num_devices=NCORES, num_swdge_queues=NQr,
                   dynamic_dma_scratch_size=scratch)
    # Leave the top of SBUF for the runtime's SWDGE descriptor rings /
    # DynamicDMAScratch carveout — allocating into it wedges the device.
    nc.sbuf_top = min(nc.sbuf_top, 192 * 1024)

    def din(name, shape, dt=f32):
        return nc.dram_tensor(name, shape, dt, kind="ExternalInput")

    fuse = bool(os.environ.get("KFUSE"))
    xt_d = din("xt", [D, N2], bf16); xbt_d = din("xbt", [D, NB2], bf16)
    wxt_d = din("wxt", [D, D], bf16); wbt_d = din("wbt", [D, D], bf16)
    w1t_d = din("w1t", [D, D])
    linbb_d = din("linbb", [P, D]); iotab_d = din("iotab", [P, GCHUNK * P], bf16)
    xct_d = din("xct", [D, span])
    chunks = _chunks(B)
    prime = min(PRIME, len(chunks))
    EP = sum(ch[2] for ch in chunks[:prime]) * P
    combw_d = din("combw", [128, 2 * (S - EP) // 16], i16)
    srcwp_d = din("srcwp", [128, EP // 16], i16)
    briwp_d = din("briwp", [128, EP // 16], i16)
    dstloc_d = din("dstloc", [P, SB], bf16)
    if fuse:
        w2t_d = din("w2t", [D, D])
        g1c_d = din("g1c", [P, 1]); b1c_d = din("b1c", [P, 1])
        g2c_d = din("g2c", [P, 1]); b2c_d = din("b2c", [P, 1])
        maskb_d = din("maskb", [P, span]); ident_d = din("ident", [P, P])
        out_d = nc.dram_tensor("out", [span, D], f32, kind="ExternalOutput")
        bn_in = {i: nc.dram_tensor(f"bn_in{i}", [P, 2], f32) for i in (1, 2)}
        bn_out = {i: nc.dram_tensor(f"bn_out{i}", [P, 2], f32) for i in (1, 2)}
    else:
        h_out_d = nc.dram_tensor("h_out", [P, span], f32, kind="ExternalOutput")
        stat_out_d = nc.dram_tensor("stat_out", [P, 2], f32,
                                    kind="ExternalOutput")

    ct_t = nc.dram_tensor("ct_tab", [N2 + NB2, D], bf16)
    with tile.TileContext(nc) as tc:
        with (
            tc.tile_pool(name="consts", bufs=1) as cp,
            tc.tile_pool(name="pa_src", bufs=2) as pa_src,
            tc.tile_pool(name="pa_stg", bufs=2) as pa_stg,
            tc.tile_pool(name="psA", bufs=2, space="PSUM") as psA,
            tc.tile_pool(name="psB", bufs=2, space="PSUM") as psB,
            tc.tile_pool(name="gx", bufs=GBUFS) as gxp,
            tc.tile_pool(name="px", bufs=max(1, min(PRIME, 20))) as pxp,
            tc.tile_pool(name="pe", bufs=4) as pep,
            tc.tile_pool(name="oh", bufs=GBUFS) as ohp,
            tc.tile_pool(name="full", bufs=1) as fullp,
            tc.tile_pool(name="small", bufs=1) as smallp,
        ):
            def load_const(name, dram, shape, dt=f32, eng=None):
                t = cp.tile(shape, dt, tag=f"c_{name}")
                (eng or nc.sync).dma_start(t[:], dram[:])
                return t

            wxt = load_const("wxt", wxt_d, [D, D], bf16)
            wbt = load_const("wbt", wbt_d, [D, D], bf16)
            w1t = load_const("w1t", w1t_d, [D, D], eng=nc.scalar)
            linbb = load_const("linbb", linbb_d, [P, D])
            iotab = load_const("iotab", iotab_d, [P, GCHUNK * P], bf16)
            xct = load_const("xct", xct_d, [D, span], eng=nc.scalar)
            combw = load_const("combw", combw_d,
                               [128, 2 * (S - EP) // 16], i16, eng=nc.scalar)
            srcwp = load_const("srcwp", srcwp_d, [128, EP // 16], i16,
                               eng=nc.scalar)
            briwp = load_const("briwp", briwp_d, [128, EP // 16], i16,
                               eng=nc.scalar)
            dstloc = load_const("dstloc", dstloc_d, [P, SB], bf16, eng=nc.scalar)
            if fuse:
                w2t = load_const("w2t", w2t_d, [D, D])
                g1c = load_const("g1c", g1c_d, [P, 1])
                b1c = load_const("b1c", b1c_d, [P, 1])
                g2c = load_const("g2c", g2c_d, [P, 1])
                b2c = load_const("b2c", b2c_d, [P, 1])
                maskb = load_const("maskb", maskb_d, [P, span])
                ident = load_const("ident", ident_d, [P, P])

            # ---------------- Phase A: node tables in DRAM ----------------
            CW = 2048  # source columns per chunk

            def build_table(src_dram, ncols, w_sbuf, row_off, add_bias):
                for c0 in range(0, ncols, CW):
                    w = min(CW, ncols - c0)
                    s = pa_src.tile([D, CW], bf16, tag="pa_src")
                    nc.sync.dma_start(s[:, :w], src_dram[:, c0:c0 + w])
                    g = pa_stg.tile([P, CW], bf16, tag="pa_stg")
                    for q0 in range(0, w, 512):
                        qw = min(512, w - q0)
                        ps = psA.tile([P, 512], f32, tag="psA")
                        for j in range(qw // P):
                            nc.tensor.matmul(
                                ps[:, j * P:(j + 1) * P],
                                s[:, q0 + j * P:q0 + (j + 1) * P], w_sbuf[:])
                        sl = g[:, q0:q0 + qw]
                        if add_bias:
                            lb = linbb[:].rearrange(
                                "p (b d) -> p b d", b=1).to_broadcast(
                                (P, qw // P, P))
                            nc.vector.tensor_tensor(
                                sl.rearrange("p (b c) -> p b c", c=P),
                                ps[:, :qw].rearrange("p (b c) -> p b c", c=P),
                                lb, Alu.add)
                        else:
                            nc.scalar.activation(sl, ps[:, :qw], Act.Copy)
                    r0 = row_off + c0
                    nc.scalar.dma_start(
                        ct_t[r0:r0 + w, :].rearrange("(g p) d -> p g d", p=P),
                        g[:, :w].rearrange("p (g d) -> p g d", d=P))

            build_table(xt_d, N2, wxt, 0, False)
            build_table(xbt_d, NB2, wbt, N2, True)

            # ---------------- Phase B: gather + scatter-add ----------------
            nchunks = len(chunks)
            outT = fullp.tile([P, span], f32, tag="outT")
            h1 = fullp.tile([P, span], f32, tag="h")
            gq = [0]

            # Primed prefix: x-gathers issued back-to-back; they only
            # depend on the xW half-table, so they drain while eW builds.
            pxs = []
            for k in range(prime):
                lt, done, cb, boff, _, _ = chunks[k]
                ne = cb * P
                goff = boff * (P // 16)
                px = pxp.tile([P, GCHUNK, D], bf16, tag="px")
                nc.gpsimd.dma_gather(
                    px[:, :cb, :], ct_t[0:N2, :],
                    srcwp[:, goff:goff + ne // 16],
                    ne, ne, D, single_packet=False, queue_num=gq[0] % NQr)
                gq[0] += 1
                pxs.append(px)

            aggT = None
            csoff = 0  # combined-stream batch offset for suffix chunks
            for k in range(nchunks):
                lt, done, cb, boff, first_of_tile, last_of_tile = chunks[k]
                ne = cb * P
                if k < prime:
                    pe = pep.tile([P, GCHUNK, D], bf16, tag="pe")
                    goff = boff * (P // 16)
                    nc.gpsimd.dma_gather(
                        pe[:, :cb, :], ct_t[N2:N2 + NB2, :],
                        briwp[:, goff:goff + ne // 16],
                        ne, ne, D, single_packet=False, queue_num=gq[0] % NQr)
                    gq[0] += 1
                    gx = pxs[k]
                    if first_of_tile:
                        aggT = psB.tile([P, P], f32, tag="aggT")
                    nc.vector.tensor_tensor(gx[:, :cb, :], gx[:, :cb, :],
                                            pe[:, :cb, :], Alu.add)
                else:
                    goff = 2 * csoff * (P // 16)
                    csoff += cb
                    gx = gxp.tile([P, 2 * GCHUNK, D], bf16, tag="gx")
                    nc.gpsimd.dma_gather(
                        gx[:, :2 * cb, :], ct_t[:],
                        combw[:, goff:goff + 2 * ne // 16],
                        2 * ne, 2 * ne, D, single_packet=False,
                        queue_num=gq[0] % NQr)
                    gq[0] += 1
                    if first_of_tile:
                        aggT = psB.tile([P, P], f32, tag="aggT")
                    nc.vector.tensor_tensor(gx[:, :cb, :], gx[:, :cb, :],
                                            gx[:, cb:2 * cb, :], Alu.add)
                nc.scalar.activation(gx[:, :cb, :], gx[:, :cb, :], Act.Relu)

                oh = ohp.tile([P, GCHUNK * P], bf16, tag="oh")
                dl = dstloc[:, boff:boff + cb]
                dl_rep = dl.to_broadcast((P, cb, P))
                nc.vector.tensor_tensor(
                    oh[:, :cb * P].rearrange("p (b c) -> p b c", c=P),
                    iotab[:, :cb * P].rearrange("p (b c) -> p b c", c=P),
                    dl_rep, Alu.is_equal)
                for b in range(cb):
                    nc.tensor.matmul(
                        aggT[:], gx[:, b, :], oh[:, b * P:(b + 1) * P],
                        start=(first_of_tile and b == 0),
                        stop=(last_of_tile and b == cb - 1))

                if last_of_tile:
                    # residual + first MLP linear for this finished tile
                    sl = slice(lt * P, (lt + 1) * P)
                    nc.vector.scalar_tensor_tensor(
                        outT[:, sl], xct[:, sl], float(resid_scale), aggT[:],
                        Alu.mult, Alu.add)
                    ps = psA.tile([P, P], f32, tag="psA")
                    nc.tensor.matmul(ps[:], w1t[:], outT[:, sl])
                    nc.scalar.activation(h1[:, sl], ps[:], Act.Copy)

            # ---------------- Phase C: MLP + BN (feature-major) ----------------
            def bn_layer(idx, h_full, gcol, bcol, apply_mask, skip_coll=False):
                s_stat = smallp.tile([P, 2], f32, tag=f"stat{idx}")
                nc.vector.reduce_sum(s_stat[:, 0:1], h_full[:],
                                     axis=mybir.AxisListType.X)
                sq = fullp.tile([P, span], f32, tag="sq")
                nc.vector.tensor_tensor(sq[:], h_full[:], h_full[:], Alu.mult)
                nc.vector.reduce_sum(s_stat[:, 1:2], sq[:],
                                     axis=mybir.AxisListType.X)
                nc.sync.dma_start(bn_in[idx][:], s_stat[:])
                if not skip_coll:
                    nc.gpsimd.collective_compute(
                        "AllReduce", Alu.add,
                        replica_groups=[list(range(NCORES))],
                        ins=[bn_in[idx][:]], outs=[bn_out[idx][:]])
                red = smallp.tile([P, 2], f32, tag=f"red{idx}")
                nc.sync.dma_start(red[:], bn_in[idx][:] if skip_coll else bn_out[idx][:])
                mu = smallp.tile([P, 1], f32, tag=f"mu{idx}")
                nc.vector.tensor_scalar(mu[:], red[:, 0:1], 1.0 / N, None, Alu.mult)
                var = smallp.tile([P, 1], f32, tag=f"var{idx}")
                mu2 = smallp.tile([P, 1], f32, tag=f"mu2{idx}")
                nc.vector.tensor_tensor(mu2[:], mu[:], mu[:], Alu.mult)
                e2 = smallp.tile([P, 1], f32, tag=f"e2{idx}")
                nc.vector.tensor_scalar(e2[:], red[:, 1:2], 1.0 / N, None, Alu.mult)
                nc.vector.tensor_tensor(var[:], e2[:], mu2[:], Alu.subtract)
                vep = smallp.tile([P, 1], f32, tag=f"vep{idx}")
                nc.vector.tensor_scalar(vep[:], var[:], BN_EPS, None, Alu.add)
                std = smallp.tile([P, 1], f32, tag=f"std{idx}")
                nc.scalar.activation(std[:], vep[:], Act.Sqrt)
                rstd = smallp.tile([P, 1], f32, tag=f"rstd{idx}")
                nc.vector.reciprocal(rstd[:], std[:])
                a = smallp.tile([P, 1], f32, tag=f"a{idx}")
                nc.vector.tensor_tensor(a[:], gcol[:], rstd[:], Alu.mult)
                sh = smallp.tile([P, 1], f32, tag=f"sh{idx}")
                tmp = smallp.tile([P, 1], f32, tag=f"tmp{idx}")
                nc.vector.tensor_tensor(tmp[:], mu[:], a[:], Alu.mult)
                nc.vector.tensor_tensor(sh[:], bcol[:], tmp[:], Alu.subtract)
                hn = fullp.tile([P, span], f32, tag="hn")
                nc.scalar.activation(hn[:], h_full[:], Act.Relu,
                                     bias=sh[:], scale=a[:])
                if apply_mask:
                    nc.vector.tensor_tensor(hn[:], hn[:], maskb[:], Alu.mult)
                return hn

            def linear(w_sbuf, src_full, idx):
                h = fullp.tile([P, span], f32, tag="h")
                for lt in range(NTC):
                    sl = slice(lt * P, (lt + 1) * P)
                    ps = psA.tile([P, P], f32, tag="psA")
                    nc.tensor.matmul(ps[:], w_sbuf[:], src_full[:, sl])
                    nc.scalar.activation(h[:, sl], ps[:], Act.Copy)
                return h

            if fuse:
                hn1 = bn_layer(1, h1, g1c, b1c, apply_mask=True)
                h2 = linear(w2t, hn1, 2)
                hn2 = bn_layer(2, h2, g2c, b2c, apply_mask=False)
                stg = fullp.tile([P, span], f32, tag="stg")
                for lt in range(NTC):
                    sl = slice(lt * P, (lt + 1) * P)
                    ps = psA.tile([P, P], f32, tag="psA")
                    nc.tensor.transpose(ps[:], hn2[:, sl], ident[:])
                    nc.scalar.activation(stg[:, sl], ps[:], Act.Copy)
                nc.sync.dma_start(
                    out_d[:].rearrange("(g p) d -> p g d", p=P),
                    stg[:].rearrange("p (g d) -> p g d", d=P))
            else:
                # per-core stats of the loop-computed h1; the 8-way BN stat
                # reduction happens on the host between NEFFs
                s_stat = smallp.tile([P, 2], f32, tag="stat1")
                nc.vector.reduce_sum(s_stat[:, 0:1], h1[:],
                                     axis=mybir.AxisListType.X)
                sq = fullp.tile([P, span], f32, tag="sq")
                nc.vector.tensor_tensor(sq[:], h1[:], h1[:], Alu.mult)
                nc.vector.reduce_sum(s_stat[:, 1:2], sq[:],
                                     axis=mybir.AxisListType.X)
                nc.sync.dma_start(stat_out_d[:], s_stat[:])
                nc.sync.dma_start(h_out_d[:], h1[:])

    nc.compile()
    return nc




_cache2 = {}


def _bn_scale_shift(nc, mybir, smallp, red, gcol, bcol, idx):
    """Device-side BN coefficients from reduced stats: a = g*rstd, sh = b - mu*a."""
    f32 = mybir.dt.float32
    Alu = mybir.AluOpType
    Act = mybir.ActivationFunctionType
    mu = smallp.tile([P, 1], f32, tag=f"mu{idx}")
    nc.vector.tensor_scalar(mu[:], red[:, 0:1], 1.0 / N, None, Alu.mult)
    mu2 = smallp.tile([P, 1], f32, tag=f"mu2{idx}")
    nc.vector.tensor_tensor(mu2[:], mu[:], mu[:], Alu.mult)
    e2 = smallp.tile([P, 1], f32, tag=f"e2{idx}")
    nc.vector.tensor_scalar(e2[:], red[:, 1:2], 1.0 / N, None, Alu.mult)
    var = smallp.tile([P, 1], f32, tag=f"var{idx}")
    nc.vector.tensor_tensor(var[:], e2[:], mu2[:], Alu.subtract)
    vep = smallp.tile([P, 1], f32, tag=f"vep{idx}")
    nc.vector.tensor_scalar(vep[:], var[:], BN_EPS, None, Alu.add)
    std = smallp.tile([P, 1], f32, tag=f"std{idx}")
    nc.scalar.activation(std[:], vep[:], Act.Sqrt)
    rstd = smallp.tile([P, 1], f32, tag=f"rstd{idx}")
    nc.vector.reciprocal(rstd[:], std[:])
    a = smallp.tile([P, 1], f32, tag=f"a{idx}")
    nc.vector.tensor_tensor(a[:], gcol[:], rstd[:], Alu.mult)
    tmp = smallp.tile([P, 1], f32, tag=f"tmp{idx}")
    nc.vector.tensor_tensor(tmp[:], mu[:], a[:], Alu.mult)
    sh = smallp.tile([P, 1], f32, tag=f"sh{idx}")
    nc.vector.tensor_tensor(sh[:], bcol[:], tmp[:], Alu.subtract)
    return a, sh


def _build_phase2():
    """NEFF2: h1n = mask*relu(BN1(h1)); h2 = h1n @ w2.T; per-core stats of h2."""
    import concourse.bacc as bacc
    import concourse.mybir as mybir
    import concourse.tile as tile

    f32 = mybir.dt.float32
    Alu = mybir.AluOpType
    Act = mybir.ActivationFunctionType
    span = NTC * P

    nc = bacc.Bacc("TRN2", target_bir_lowering=False, debug=False,
                   num_devices=NCORES)
    nc.sbuf_top = min(nc.sbuf_top, 192 * 1024)

    def din(name, shape):
        return nc.dram_tensor(name, shape, f32, kind="ExternalInput")

    h_d = din("h_in", [P, span])
    red_d = din("red", [P, 2])
    w2t_d = din("w2t", [D, D])
    g1c_d = din("g1c", [P, 1]); b1c_d = din("b1c", [P, 1])
    maskb_d = din("maskb", [P, span])
    h_out_d = nc.dram_tensor("h_out", [P, span], f32, kind="ExternalOutput")
    stat_out_d = nc.dram_tensor("stat_out", [P, 2], f32, kind="ExternalOutput")

    with tile.TileContext(nc) as tc:
        with (
            tc.tile_pool(name="consts", bufs=1) as cp,
            tc.tile_pool(name="psA", bufs=2, space="PSUM") as psA,
            tc.tile_pool(name="full", bufs=1) as fullp,
            tc.tile_pool(name="small", bufs=1) as smallp,
        ):
            def load_const(name, dram, shape):
                t = cp.tile(shape, f32, tag=f"c_{name}")
                nc.sync.dma_start(t[:], dram[:])
                return t

            h1 = load_const("h", h_d, [P, span])
            red = load_const("red", red_d, [P, 2])
            w2t = load_const("w2t", w2t_d, [D, D])
            g1c = load_const("g1c", g1c_d, [P, 1])
            b1c = load_const("b1c", b1c_d, [P, 1])
            maskb = load_const("maskb", maskb_d, [P, span])

            a, sh = _bn_scale_shift(nc, mybir, smallp, red, g1c, b1c, 0)
            hn = fullp.tile([P, span], f32, tag="hn")
            nc.scalar.activation(hn[:], h1[:], Act.Relu, bias=sh[:], scale=a[:])
            nc.vector.tensor_tensor(hn[:], hn[:], maskb[:], Alu.mult)

            h2 = fullp.tile([P, span], f32, tag="h2")
            for lt in range(NTC):
                sl = slice(lt * P, (lt + 1) * P)
                ps = psA.tile([P, P], f32, tag="psA")
                nc.tensor.matmul(ps[:], w2t[:], hn[:, sl])
                nc.scalar.activation(h2[:, sl], ps[:], Act.Copy)
            s_stat = smallp.tile([P, 2], f32, tag="stat2")
            nc.vector.reduce_sum(s_stat[:, 0:1], h2[:],
                                 axis=mybir.AxisListType.X)
            sq = fullp.tile([P, span], f32, tag="sq")
            nc.vector.tensor_tensor(sq[:], h2[:], h2[:], Alu.mult)
            nc.vector.reduce_sum(s_stat[:, 1:2], sq[:],
                                 axis=mybir.AxisListType.X)
            nc.sync.dma_start(stat_out_d[:], s_stat[:])
            nc.sync.dma_start(h_out_d[:], h2[:])

    nc.compile()
    return nc


def _build_phase3():
    """NEFF3: out = transpose(relu(BN2(h2)))."""
    import concourse.bacc as bacc
    import concourse.mybir as mybir
    import concourse.tile as tile

    f32 = mybir.dt.float32
    Act = mybir.ActivationFunctionType
    span = NTC * P

    nc = bacc.Bacc("TRN2", target_bir_lowering=False, debug=False,
                   num_devices=NCORES)
    nc.sbuf_top = min(nc.sbuf_top, 192 * 1024)

    def din(name, shape):
        return nc.dram_tensor(name, shape, f32, kind="ExternalInput")

    h_d = din("h_in", [P, span])
    red_d = din("red", [P, 2])
    g2c_d = din("g2c", [P, 1]); b2c_d = din("b2c", [P, 1])
    ident_d = din("ident", [P, P])
    out_d = nc.dram_tensor("out", [span, D], f32, kind="ExternalOutput")

    with tile.TileContext(nc) as tc:
        with (
            tc.tile_pool(name="consts", bufs=1) as cp,
            tc.tile_pool(name="psA", bufs=2, space="PSUM") as psA,
            tc.tile_pool(name="full", bufs=1) as fullp,
            tc.tile_pool(name="small", bufs=1) as smallp,
        ):
            def load_const(name, dram, shape):
                t = cp.tile(shape, f32, tag=f"c_{name}")
                nc.sync.dma_start(t[:], dram[:])
                return t

            h2 = load_const("h", h_d, [P, span])
            red = load_const("red", red_d, [P, 2])
            g2c = load_const("g2c", g2c_d, [P, 1])
            b2c = load_const("b2c", b2c_d, [P, 1])
            ident = load_const("ident", ident_d, [P, P])

            a, sh = _bn_scale_shift(nc, mybir, smallp, red, g2c, b2c, 1)
            hn = fullp.tile([P, span], f32, tag="hn")
            nc.scalar.activation(hn[:], h2[:], Act.Relu, bias=sh[:], scale=a[:])

            stg = fullp.tile([P, span], f32, tag="stg")
            for lt in range(NTC):
                sl = slice(lt * P, (lt + 1) * P)
                ps = psA.tile([P, P], f32, tag="psA")
                nc.tensor.transpose(ps[:], hn[:, sl], ident[:])
                nc.scalar.activation(stg[:, sl], ps[:], Act.Copy)
            nc.sync.dma_start(
                out_d[:].rearrange("(g p) d -> p g d", p=P),
                stg[:].rearrange("p (g d) -> p g d", d=P))

    nc.compile()
    return nc


def kernel(**inputs):
    global last_results
    from concourse.bass_utils import run_bass_kernel_spmd

    in_maps, meta = _host_prep(inputs)
    if meta not in _cache:
        _cache[meta] = _build(meta)
    cores = list(range(NCORES))
    trace = bool(os.environ.get("KERNEL_TRACE"))

    if os.environ.get("KFUSE"):
        nc1 = _cache[meta]
        k1 = ("xt", "xbt", "wxt", "wbt", "w1t", "linbb", "iotab", "xct",
              "combw", "srcwp", "briwp", "dstloc", "w2t", "g1c", "b1c", "g2c", "b2c",
              "maskb", "ident")
        in1 = [{k: in_maps[c][k] for k in k1} for c in range(NCORES)]
        res1 = run_bass_kernel_spmd(nc1, in1, cores, trace=trace)
        last_results = (res1,)
        out = np.concatenate([res1.results[c]["out"]
                              for c in range(NCORES)], axis=0)
        return np.ascontiguousarray(out[:N])

    if "p2" not in _cache2:
        _cache2["p2"] = _build_phase2()
        _cache2["p3"] = _build_phase3()
    nc1, nc2, nc3 = _cache[meta], _cache2["p2"], _cache2["p3"]

    k1 = ("xt", "xbt", "wxt", "wbt", "w1t", "linbb", "iotab", "xct",
          "combw", "srcwp", "briwp", "dstloc")
    in1 = [{k: in_maps[c][k] for k in k1} for c in range(NCORES)]
    res1 = run_bass_kernel_spmd(nc1, in1, cores, trace=trace)
    red1 = np.sum([res1.results[c]["stat_out"] for c in range(NCORES)], axis=0)
    in2 = [{"h_in": res1.results[c]["h_out"], "red": red1,
            "w2t": in_maps[c]["w2t"], "g1c": in_maps[c]["g1c"],
            "b1c": in_maps[c]["b1c"], "maskb": in_maps[c]["maskb"]}
           for c in range(NCORES)]
    res2 = run_bass_kernel_spmd(nc2, in2, cores, trace=trace)
    red2 = np.sum([res2.results[c]["stat_out"] for c in range(NCORES)], axis=0)
    in3 = [{"h_in": res2.results[c]["h_out"], "red": red2,
            "g2c": in_maps[c]["g2c"], "b2c": in_maps[c]["b2c"],
            "ident": in_maps[c]["ident"]} for c in range(NCORES)]
    res3 = run_bass_kernel_spmd(nc3, in3, cores, trace=trace)

    last_results = (res1, res2, res3)
    out = np.concatenate([res3.results[c]["out"] for c in range(NCORES)], axis=0)
    return np.ascontiguousarray(out[:N])



# revision 43
# speedup vs baseline: 1.1414x; 1.1414x over previous
"""Trainium2 Bass kernel for nn_AdjacencyConv (GNN message passing).

Reference computation:
    msg  = relu(concat[x[src], x_bridge[bri]] @ lin_w.T + lin_b)   # [E, D]
    agg  = segment_sum(msg, dst, N)                                # [N, D]
    out  = agg + (1+eps)*x
    h    = relu(BN(out @ w1.T + b1)); h = relu(BN(h @ w2.T + b2))  # train-mode BN

Device algorithm (8-core SPMD, edges sharded by dst node-tile):
  Phase A (per core, replicated): build bf16 node tables in DRAM
        xW = x @ Wx.T            (Wx = lin_w[:, :D])      rows [0, N2)
        eW = x_bridge @ Wb.T + b (Wb = lin_w[:, D:])      rows [N2, N2+Nb2)
    so the per-edge linear factorizes: msg = relu(xW[src] + eW[bri]).
  Phase B: per dst node-tile of 128 nodes, one combined dma_gather per
    chunk fetches both rows of each edge from the concatenated table
    (edge-major bf16 [128e, D]), add + relu, then scatter-add via one-hot
    matmuls accumulating feature-major agg fp32 in PSUM. Residual and the
    first MLP linear run per-tile inside the loop. The first PRIME chunks
    instead use split x/e gathers: their x-gathers depend only on the xW
    half-table, so they start draining while eW still builds (hides most
    of Phase A). Phase B is bound by the SWDGE drain rate (~2.5 ns per
    descriptor across the 4 queues); bf16 halves gather bytes and gives
    one-pass PE matmuls vs fp32.
  Tail: 2 small follow-up NEFFs apply train-mode BN (biases b1/b2 cancel
    in BN and are dropped); the [128, 2] BN stat reduction across cores is
    host-stitched between NEFFs — a device AllReduce measured ~200us worse
    because it absorbs cross-core launch skew into the NEFF exec time.

Host side does only layout preprocessing: index sorting/padding/packing,
weight transposes, the 2KB BN stat reductions, and output assembly.
"""

import os
import numpy as np
import ml_dtypes

BF16 = ml_dtypes.bfloat16
N, NB, E, D = 10000, 20000, 640000, 128
P = 128
NCORES = 8
NTC = 10                    # node tiles per core
N2 = NCORES * NTC * P       # 10240 padded nodes
NB2 = 20096                 # x_bridge padded to multiple of 128
BN_EPS = 1e-5
GCHUNK = int(os.environ.get("KGCHUNK", "8"))  # batches per dma_gather
NQ = int(os.environ.get("KNQ", "4"))   # SWDGE queues
GBUFS = int(os.environ.get("KGBUFS", "10"))  # gather tile-pool depth
PRIME = int(os.environ.get("KPRIME", "20"))  # split-gather prefix chunks

_cache = {}

# exposed for test.py
last_results = None


def _chunks(B):
    """Chunk enumeration shared by host prep and kernel build:
    (lt, done, cb, batch_off, first_of_tile, last_of_tile)."""
    out = []
    gbase = 0
    for lt in range(NTC):
        nb = int(B[lt])
        done = 0
        while done < nb:
            cb = min(GCHUNK, nb - done)
            out.append((lt, done, cb, gbase + done,
                        done == 0, done + cb == nb))
            done += cb
        gbase += nb
    return out


def _pack_idx(idx16):
    """Wrap an int16 index vector for dma_gather: [16, n/16] replicated x8."""
    w = idx16.reshape(-1, 16).T
    return np.tile(w, (8, 1)).copy()


def _host_prep(inputs):
    x = np.asarray(inputs["x"], np.float32)
    xb = np.asarray(inputs["x_bridge"], np.float32)
    ei = np.asarray(inputs["edge_index"])
    bri = np.asarray(inputs["bridge_index"])
    lin_w = np.asarray(inputs["lin_w"], np.float32)
    lin_b = np.asarray(inputs["lin_b"], np.float32)
    eps = float(np.asarray(inputs["eps"]).reshape(-1)[0])
    w1 = np.asarray(inputs["w1"], np.float32)
    g1 = np.asarray(inputs["g1"], np.float32)
    beta1 = np.asarray(inputs["beta1"], np.float32)
    w2 = np.asarray(inputs["w2"], np.float32)
    g2 = np.asarray(inputs["g2"], np.float32)
    beta2 = np.asarray(inputs["beta2"], np.float32)

    src = ei[0].astype(np.int64)
    dst = ei[1].astype(np.int64)
    bri = bri.astype(np.int64)

    # sort edges by dst, bucket into 128-node tiles
    order = np.argsort(dst, kind="stable")
    dsts = dst[order]
    srcs = src[order]
    bris = bri[order]
    gt_bounds = np.searchsorted(dsts, np.arange(NCORES * NTC + 1) * P)

    # uniform program structure: per local tile lt, same batch count across cores
    cnt = np.diff(gt_bounds)  # edges per global tile (len 80)
    cnt = cnt.reshape(NCORES, NTC)
    ceils = -(-cnt // P)
    B = np.maximum(1, ceils.max(axis=0))  # batches per local tile (len NTC)
    SB = int(B.sum())          # total batches per core
    S = SB * P                 # padded edges per core

    src_pad = np.zeros((NCORES, S), np.int64)
    bri_pad = np.zeros((NCORES, S), np.int64)
    dloc_pad = np.full((NCORES, S), 999.0, np.float32)
    for c in range(NCORES):
        off = 0
        for lt in range(NTC):
            gt = c * NTC + lt
            a, b = gt_bounds[gt], gt_bounds[gt + 1]
            n = b - a
            src_pad[c, off:off + n] = srcs[a:b]
            bri_pad[c, off:off + n] = bris[a:b]
            dloc_pad[c, off:off + n] = (dsts[a:b] - gt * P).astype(np.float32)
            off += B[lt] * P
    # dstloc transposed: [128, SB], column j = batch j's 128 local-dst values
    dloc_T = np.ascontiguousarray(
        dloc_pad.reshape(NCORES, SB, P).transpose(0, 2, 1)).astype(BF16)

    # Gather index streams. The first `prime` chunks use split x/e gathers
    # (x-gathers only depend on the xW half-table, so they start while eW
    # still builds); the rest use one combined gather per chunk:
    # [src idx (cb*128), bri idx + N2 (cb*128)].
    chunks = _chunks(B)
    prime = min(PRIME, len(chunks))
    ep = sum(ch[2] for ch in chunks[:prime]) * P  # prefix edges
    comb = np.zeros((NCORES, 2 * (S - ep)), np.int64)
    for c in range(NCORES):
        woff = 0
        for (lt, done, cb, boff, _, _) in chunks[prime:]:
            off = boff * P
            ne = cb * P
            comb[c, woff:woff + ne] = src_pad[c, off:off + ne]
            comb[c, woff + ne:woff + 2 * ne] = bri_pad[c, off:off + ne] + N2
            woff += 2 * ne



    # feature-major padded inputs for the table build (bf16)
    xt = np.zeros((D, N2), BF16)
    xt[:, :N] = x.T.astype(BF16)
    xbt = np.zeros((D, NB2), BF16)
    xbt[:, :NB] = xb.T.astype(BF16)

    wxt = np.ascontiguousarray(lin_w[:, :D].T).astype(BF16)   # [in_f, out]
    wbt = np.ascontiguousarray(lin_w[:, D:].T).astype(BF16)
    w1t = np.ascontiguousarray(w1.T)
    w2t = np.ascontiguousarray(w2.T)
    linbb = np.tile(lin_b[None, :], (P, 1)).astype(np.float32)   # [128, 128]
    iotab = np.tile(np.arange(P, dtype=np.float32)[None, :], (P, GCHUNK)).astype(BF16)
    ident = np.eye(P, dtype=np.float32)

    g1c = np.ascontiguousarray(g1[:, None])
    b1c = np.ascontiguousarray(beta1[:, None])
    g2c = np.ascontiguousarray(g2[:, None])
    b2c = np.ascontiguousarray(beta2[:, None])

    # per-core residual slice (feature-major) and validity mask
    span = NTC * P
    xct = np.zeros((NCORES, D, span), np.float32)
    maskb = np.zeros((NCORES, P, span), np.float32)
    for c in range(NCORES):
        c0 = c * span
        v = min(max(N - c0, 0), span)
        if v > 0:
            xct[c, :, :v] = x.T[:, c0:c0 + v]
            maskb[c, :, :v] = 1.0

    in_maps = []
    for c in range(NCORES):
        in_maps.append({
            "xt": xt, "xbt": xbt,
            "wxt": wxt, "wbt": wbt, "w1t": w1t, "w2t": w2t,
            "linbb": linbb, "iotab": iotab, "ident": ident,
            "g1c": g1c, "b1c": b1c, "g2c": g2c, "b2c": b2c,
            "xct": np.ascontiguousarray(xct[c]),
            "maskb": np.ascontiguousarray(maskb[c]),
            "combw": _pack_idx(comb[c].astype(np.int16)),
            "srcwp": _pack_idx(src_pad[c, :ep].astype(np.int16)),
            "briwp": _pack_idx(bri_pad[c, :ep].astype(np.int16)),
            "dstloc": np.ascontiguousarray(dloc_T[c]),
        })
    meta = (tuple(int(b) for b in B), 1.0 + eps)
    return in_maps, meta


def _build(meta):
    import concourse.bacc as bacc
    import concourse.mybir as mybir
    import concourse.tile as tile

    KP = 3  # full fused kernel
    B, resid_scale = meta
    SB = sum(B)
    S = SB * P
    f32 = mybir.dt.float32
    bf16 = mybir.dt.bfloat16
    i16 = mybir.dt.int16
    Alu = mybir.AluOpType
    Act = mybir.ActivationFunctionType
    span = NTC * P

    NQr = int(os.environ.get("KNQ", "4"))
    scratch = int(os.environ.get("KSCRATCH", "16384"))
    nc = bacc.Bacc("TRN2", target_bir_lowering=False, debug=False,
                   num_devices=NCORES, num_swdge_queues=NQr,
                   dynamic_dma_scratch_size=scratch)
    # Leave the top of SBUF for the runtime's SWDGE descriptor rings /
    # DynamicDMAScratch carveout — allocating into it wedges the device.
    nc.sbuf_top = min(nc.sbuf_top, 192 * 1024)

    def din(name, shape, dt=f32):
        return nc.dram_tensor(name, shape, dt, kind="ExternalInput")

    fuse = bool(os.environ.get("KFUSE"))
    xt_d = din("xt", [D, N2], bf16); xbt_d = din("xbt", [D, NB2], bf16)
    wxt_d = din("wxt", [D, D], bf16); wbt_d = din("wbt", [D, D], bf16)
    w1t_d = din("w1t", [D, D])
    linbb_d = din("linbb", [P, D]); iotab_d = din("iotab", [P, GCHUNK * P], bf16)
    xct_d = din("xct", [D, span])
    chunks = _chunks(B)
    prime = min(PRIME, len(chunks))
    EP = sum(ch[2] for ch in chunks[:prime]) * P
    combw_d = din("combw", [128, 2 * (S - EP) // 16], i16)
    srcwp_d = din("srcwp", [128, EP // 16], i16)
    briwp_d = din("briwp", [128, EP // 16], i16)
    dstloc_d = din("dstloc", [P, SB], bf16)
    if fuse:
        w2t_d = din("w2t", [D, D])
        g1c_d = din("g1c", [P, 1]); b1c_d = din("b1c", [P, 1])
        g2c_d = din("g2c", [P, 1]); b2c_d = din("b2c", [P, 1])
        maskb_d = din("maskb", [P, span]); ident_d = din("ident", [P, P])
        out_d = nc.dram_tensor("out", [span, D], f32, kind="ExternalOutput")
        bn_in = {i: nc.dram_tensor(f"bn_in{i}", [P, 2], f32) for i in (1, 2)}
        bn_out = {i: nc.dram_tensor(f"bn_out{i}", [P, 2], f32) for i in (1, 2)}
    else:
        h_out_d = nc.dram_tensor("h_out", [P, span], f32, kind="ExternalOutput")
        stat_out_d = nc.dram_tensor("stat_out", [P, 2], f32,
                                    kind="ExternalOutput")

    ct_t = nc.dram_tensor("ct_tab", [N2 + NB2, D], bf16)
    with tile.TileContext(nc) as tc:
        with (
            tc.tile_pool(name="consts", bufs=1) as cp,
            tc.tile_pool(name="pa_src", bufs=2) as pa_src,
            tc.tile_pool(name="pa_stg", bufs=2) as pa_stg,
            tc.tile_pool(name="psA", bufs=2, space="PSUM") as psA,
            tc.tile_pool(name="psB", bufs=2, space="PSUM") as psB,
            tc.tile_pool(name="gx", bufs=GBUFS) as gxp,
            tc.tile_pool(name="px", bufs=max(1, min(PRIME, 20))) as pxp,
            tc.tile_pool(name="pe", bufs=4) as pep,
            tc.tile_pool(name="oh", bufs=GBUFS) as ohp,
            tc.tile_pool(name="full", bufs=1) as fullp,
            tc.tile_pool(name="small", bufs=1) as smallp,
        ):
            def load_const(name, dram, shape, dt=f32, eng=None):
                t = cp.tile(shape, dt, tag=f"c_{name}")
                (eng or nc.sync).dma_start(t[:], dram[:])
                return t

            wxt = load_const("wxt", wxt_d, [D, D], bf16)
            wbt = load_const("wbt", wbt_d, [D, D], bf16)
            w1t = load_const("w1t", w1t_d, [D, D], eng=nc.scalar)
            linbb = load_const("linbb", linbb_d, [P, D])
            iotab = load_const("iotab", iotab_d, [P, GCHUNK * P], bf16)
            xct = load_const("xct", xct_d, [D, span], eng=nc.scalar)
            combw = load_const("combw", combw_d,
                               [128, 2 * (S - EP) // 16], i16, eng=nc.scalar)
            srcwp = load_const("srcwp", srcwp_d, [128, EP // 16], i16,
                               eng=nc.scalar)
            briwp = load_const("briwp", briwp_d, [128, EP // 16], i16,
                               eng=nc.scalar)
            dstloc = load_const("dstloc", dstloc_d, [P, SB], bf16, eng=nc.scalar)
            if fuse:
                w2t = load_const("w2t", w2t_d, [D, D])
                g1c = load_const("g1c", g1c_d, [P, 1])
                b1c = load_const("b1c", b1c_d, [P, 1])
                g2c = load_const("g2c", g2c_d, [P, 1])
                b2c = load_const("b2c", b2c_d, [P, 1])
                maskb = load_const("maskb", maskb_d, [P, span])
                ident = load_const("ident", ident_d, [P, P])

            # ---------------- Phase A: node tables in DRAM ----------------
            CW = 2048  # source columns per chunk

            def build_table(src_dram, ncols, w_sbuf, row_off, add_bias):
                for c0 in range(0, ncols, CW):
                    w = min(CW, ncols - c0)
                    s = pa_src.tile([D, CW], bf16, tag="pa_src")
                    nc.sync.dma_start(s[:, :w], src_dram[:, c0:c0 + w])
                    g = pa_stg.tile([P, CW], bf16, tag="pa_stg")
                    for q0 in range(0, w, 512):
                        qw = min(512, w - q0)
                        ps = psA.tile([P, 512], f32, tag="psA")
                        for j in range(qw // P):
                            nc.tensor.matmul(
                                ps[:, j * P:(j + 1) * P],
                                s[:, q0 + j * P:q0 + (j + 1) * P], w_sbuf[:])
                        sl = g[:, q0:q0 + qw]
                        if add_bias:
                            lb = linbb[:].rearrange(
                                "p (b d) -> p b d", b=1).to_broadcast(
                                (P, qw // P, P))
                            nc.vector.tensor_tensor(
                                sl.rearrange("p (b c) -> p b c", c=P),
                                ps[:, :qw].rearrange("p (b c) -> p b c", c=P),
                                lb, Alu.add)
                        else:
                            nc.scalar.activation(sl, ps[:, :qw], Act.Copy)
                    r0 = row_off + c0
                    nc.scalar.dma_start(
                        ct_t[r0:r0 + w, :].rearrange("(g p) d -> p g d", p=P),
                        g[:, :w].rearrange("p (g d) -> p g d", d=P))

            build_table(xt_d, N2, wxt, 0, False)
            build_table(xbt_d, NB2, wbt, N2, True)

            # ---------------- Phase B: gather + scatter-add ----------------
            nchunks = len(chunks)
            outT = fullp.tile([P, span], f32, tag="outT")
            h1 = fullp.tile([P, span], f32, tag="h")
            gq = [0]

            # Primed prefix: x-gathers issued back-to-back; they only
            # depend on the xW half-table, so they drain while eW builds.
            pxs = []
            for k in range(prime):
                lt, done, cb, boff, _, _ = chunks[k]
                ne = cb * P
                goff = boff * (P // 16)
                px = pxp.tile([P, GCHUNK, D], bf16, tag="px")
                nc.gpsimd.dma_gather(
                    px[:, :cb, :], ct_t[0:N2, :],
                    srcwp[:, goff:goff + ne // 16],
                    ne, ne, D, single_packet=False, queue_num=gq[0] % NQr)
                gq[0] += 1
                pxs.append(px)

            aggT = None
            csoff = 0  # combined-stream batch offset for suffix chunks
            for k in range(nchunks):
                lt, done, cb, boff, first_of_tile, last_of_tile = chunks[k]
                ne = cb * P
                if k < prime:
                    pe = pep.tile([P, GCHUNK, D], bf16, tag="pe")
                    goff = boff * (P // 16)
                    nc.gpsimd.dma_gather(
                        pe[:, :cb, :], ct_t[N2:N2 + NB2, :],
                        briwp[:, goff:goff + ne // 16],
                        ne, ne, D, single_packet=False, queue_num=gq[0] % NQr)
                    gq[0] += 1
                    gx = pxs[k]
                    if first_of_tile:
                        aggT = psB.tile([P, P], f32, tag="aggT")
                    nc.vector.tensor_tensor(gx[:, :cb, :], gx[:, :cb, :],
                                            pe[:, :cb, :], Alu.add)
                else:
                    goff = 2 * csoff * (P // 16)
                    csoff += cb
                    gx = gxp.tile([P, 2 * GCHUNK, D], bf16, tag="gx")
                    nc.gpsimd.dma_gather(
                        gx[:, :2 * cb, :], ct_t[:],
                        combw[:, goff:goff + 2 * ne // 16],
                        2 * ne, 2 * ne, D, single_packet=False,
                        queue_num=gq[0] % NQr)
                    gq[0] += 1
                    if first_of_tile:
                        aggT = psB.tile([P, P], f32, tag="aggT")
                    nc.vector.tensor_tensor(gx[:, :cb, :], gx[:, :cb, :],
                                            gx[:, cb:2 * cb, :], Alu.add)
                nc.scalar.activation(gx[:, :cb, :], gx[:, :cb, :], Act.Relu)

                oh = ohp.tile([P, GCHUNK * P], bf16, tag="oh")
                dl = dstloc[:, boff:boff + cb]
                dl_rep = dl.to_broadcast((P, cb, P))
                nc.vector.tensor_tensor(
                    oh[:, :cb * P].rearrange("p (b c) -> p b c", c=P),
                    iotab[:, :cb * P].rearrange("p (b c) -> p b c", c=P),
                    dl_rep, Alu.is_equal)
                for b in range(cb):
                    nc.tensor.matmul(
                        aggT[:], gx[:, b, :], oh[:, b * P:(b + 1) * P],
                        start=(first_of_tile and b == 0),
                        stop=(last_of_tile and b == cb - 1))

                if last_of_tile:
                    # residual + first MLP linear for this finished tile
                    sl = slice(lt * P, (lt + 1) * P)
                    nc.vector.scalar_tensor_tensor(
                        outT[:, sl], xct[:, sl], float(resid_scale), aggT[:],
                        Alu.mult, Alu.add)
                    ps = psA.tile([P, P], f32, tag="psA")
                    nc.tensor.matmul(ps[:], w1t[:], outT[:, sl])
                    nc.scalar.activation(h1[:, sl], ps[:], Act.Copy)

            # ---------------- Phase C: MLP + BN (feature-major) ----------------
            def bn_layer(idx, h_full, gcol, bcol, apply_mask, skip_coll=False):
                s_stat = smallp.tile([P, 2], f32, tag=f"stat{idx}")
                nc.vector.reduce_sum(s_stat[:, 0:1], h_full[:],
                                     axis=mybir.AxisListType.X)
                sq = fullp.tile([P, span], f32, tag="sq")
                nc.vector.tensor_tensor(sq[:], h_full[:], h_full[:], Alu.mult)
                nc.vector.reduce_sum(s_stat[:, 1:2], sq[:],
                                     axis=mybir.AxisListType.X)
                nc.sync.dma_start(bn_in[idx][:], s_stat[:])
                if not skip_coll:
                    nc.gpsimd.collective_compute(
                        "AllReduce", Alu.add,
                        replica_groups=[list(range(NCORES))],
                        ins=[bn_in[idx][:]], outs=[bn_out[idx][:]])
                red = smallp.tile([P, 2], f32, tag=f"red{idx}")
                nc.sync.dma_start(red[:], bn_in[idx][:] if skip_coll else bn_out[idx][:])
                mu = smallp.tile([P, 1], f32, tag=f"mu{idx}")
                nc.vector.tensor_scalar(mu[:], red[:, 0:1], 1.0 / N, None, Alu.mult)
                var = smallp.tile([P, 1], f32, tag=f"var{idx}")
                mu2 = smallp.tile([P, 1], f32, tag=f"mu2{idx}")
                nc.vector.tensor_tensor(mu2[:], mu[:], mu[:], Alu.mult)
                e2 = smallp.tile([P, 1], f32, tag=f"e2{idx}")
                nc.vector.tensor_scalar(e2[:], red[:, 1:2], 1.0 / N, None, Alu.mult)
                nc.vector.tensor_tensor(var[:], e2[:], mu2[:], Alu.subtract)
                vep = smallp.tile([P, 1], f32, tag=f"vep{idx}")
                nc.vector.tensor_scalar(vep[:], var[:], BN_EPS, None, Alu.add)
                std = smallp.tile([P, 1], f32, tag=f"std{idx}")
                nc.scalar.activation(std[:], vep[:], Act.Sqrt)
                rstd = smallp.tile([P, 1], f32, tag=f"rstd{idx}")
                nc.vector.reciprocal(rstd[:], std[:])
                a = smallp.tile([P, 1], f32, tag=f"a{idx}")
                nc.vector.tensor_tensor(a[:], gcol[:], rstd[:], Alu.mult)
                sh = smallp.tile([P, 1], f32, tag=f"sh{idx}")
                tmp = smallp.tile([P, 1], f32, tag=f"tmp{idx}")
                nc.vector.tensor_tensor(tmp[:], mu[:], a[:], Alu.mult)
                nc.vector.tensor_tensor(sh[:], bcol[:], tmp[:], Alu.subtract)
                hn = fullp.tile([P, span], f32, tag="hn")
                nc.scalar.activation(hn[:], h_full[:], Act.Relu,
                                     bias=sh[:], scale=a[:])
                if apply_mask:
                    nc.vector.tensor_tensor(hn[:], hn[:], maskb[:], Alu.mult)
                return hn

            def linear(w_sbuf, src_full, idx):
                h = fullp.tile([P, span], f32, tag="h")
                for lt in range(NTC):
                    sl = slice(lt * P, (lt + 1) * P)
                    ps = psA.tile([P, P], f32, tag="psA")
                    nc.tensor.matmul(ps[:], w_sbuf[:], src_full[:, sl])
                    nc.scalar.activation(h[:, sl], ps[:], Act.Copy)
                return h

            if fuse:
                hn1 = bn_layer(1, h1, g1c, b1c, apply_mask=True)
                h2 = linear(w2t, hn1, 2)
                hn2 = bn_layer(2, h2, g2c, b2c, apply_mask=False)
                stg = fullp.tile([P, span], f32, tag="stg")
                for lt in range(NTC):
                    sl = slice(lt * P, (lt + 1) * P)
                    ps = psA.tile([P, P], f32, tag="psA")
                    nc.tensor.transpose(ps[:], hn2[:, sl], ident[:])
                    nc.scalar.activation(stg[:, sl], ps[:], Act.Copy)
                nc.sync.dma_start(
                    out_d[:].rearrange("(g p) d -> p g d", p=P),
                    stg[:].rearrange("p (g d) -> p g d", d=P))
            else:
                # per-core stats of the loop-computed h1; the 8-way BN stat
                # reduction happens on the host between NEFFs
                s_stat = smallp.tile([P, 2], f32, tag="stat1")
                nc.vector.reduce_sum(s_stat[:, 0:1], h1[:],
                                     axis=mybir.AxisListType.X)
                sq = fullp.tile([P, span], f32, tag="sq")
                nc.vector.tensor_tensor(sq[:], h1[:], h1[:], Alu.mult)
                nc.vector.reduce_sum(s_stat[:, 1:2], sq[:],
                                     axis=mybir.AxisListType.X)
                nc.sync.dma_start(stat_out_d[:], s_stat[:])
                nc.sync.dma_start(h_out_d[:], h1[:])

    nc.compile()
    return nc




_cache2 = {}


def _bn_scale_shift(nc, mybir, smallp, red, gcol, bcol, idx):
    """Device-side BN coefficients from reduced stats: a = g*rstd, sh = b - mu*a."""
    f32 = mybir.dt.float32
    Alu = mybir.AluOpType
    Act = mybir.ActivationFunctionType
    mu = smallp.tile([P, 1], f32, tag=f"mu{idx}")
    nc.vector.tensor_scalar(mu[:], red[:, 0:1], 1.0 / N, None, Alu.mult)
    mu2 = smallp.tile([P, 1], f32, tag=f"mu2{idx}")
    nc.vector.tensor_tensor(mu2[:], mu[:], mu[:], Alu.mult)
    e2 = smallp.tile([P, 1], f32, tag=f"e2{idx}")
    nc.vector.tensor_scalar(e2[:], red[:, 1:2], 1.0 / N, None, Alu.mult)
    var = smallp.tile([P, 1], f32, tag=f"var{idx}")
    nc.vector.tensor_tensor(var[:], e2[:], mu2[:], Alu.subtract)
    vep = smallp.tile([P, 1], f32, tag=f"vep{idx}")
    nc.vector.tensor_scalar(vep[:], var[:], BN_EPS, None, Alu.add)
    std = smallp.tile([P, 1], f32, tag=f"std{idx}")
    nc.scalar.activation(std[:], vep[:], Act.Sqrt)
    rstd = smallp.tile([P, 1], f32, tag=f"rstd{idx}")
    nc.vector.reciprocal(rstd[:], std[:])
    a = smallp.tile([P, 1], f32, tag=f"a{idx}")
    nc.vector.tensor_tensor(a[:], gcol[:], rstd[:], Alu.mult)
    tmp = smallp.tile([P, 1], f32, tag=f"tmp{idx}")
    nc.vector.tensor_tensor(tmp[:], mu[:], a[:], Alu.mult)
    sh = smallp.tile([P, 1], f32, tag=f"sh{idx}")
    nc.vector.tensor_tensor(sh[:], bcol[:], tmp[:], Alu.subtract)
    return a, sh


def _build_phase2():
    """NEFF2: h1n = mask*relu(BN1(h1)); h2 = h1n @ w2.T; per-core stats of h2."""
    import concourse.bacc as bacc
    import concourse.mybir as mybir
    import concourse.tile as tile

    f32 = mybir.dt.float32
    Alu = mybir.AluOpType
    Act = mybir.ActivationFunctionType
    span = NTC * P

    nc = bacc.Bacc("TRN2", target_bir_lowering=False, debug=False,
                   num_devices=NCORES)
    nc.sbuf_top = min(nc.sbuf_top, 192 * 1024)

    def din(name, shape):
        return nc.dram_tensor(name, shape, f32, kind="ExternalInput")

    h_d = din("h_in", [P, span])
    red_d = din("red", [P, 2])
    w2t_d = din("w2t", [D, D])
    g1c_d = din("g1c", [P, 1]); b1c_d = din("b1c", [P, 1])
    maskb_d = din("maskb", [P, span])
    h_out_d = nc.dram_tensor("h_out", [P, span], f32, kind="ExternalOutput")
    stat_out_d = nc.dram_tensor("stat_out", [P, 2], f32, kind="ExternalOutput")

    with tile.TileContext(nc) as tc:
        with (
            tc.tile_pool(name="consts", bufs=1) as cp,
            tc.tile_pool(name="psA", bufs=2, space="PSUM") as psA,
            tc.tile_pool(name="full", bufs=1) as fullp,
            tc.tile_pool(name="small", bufs=1) as smallp,
        ):
            def load_const(name, dram, shape):
                t = cp.tile(shape, f32, tag=f"c_{name}")
                nc.sync.dma_start(t[:], dram[:])
                return t

            h1 = load_const("h", h_d, [P, span])
            red = load_const("red", red_d, [P, 2])
            w2t = load_const("w2t", w2t_d, [D, D])
            g1c = load_const("g1c", g1c_d, [P, 1])
            b1c = load_const("b1c", b1c_d, [P, 1])
            maskb = load_const("maskb", maskb_d, [P, span])

            a, sh = _bn_scale_shift(nc, mybir, smallp, red, g1c, b1c, 0)
            hn = fullp.tile([P, span], f32, tag="hn")
            nc.scalar.activation(hn[:], h1[:], Act.Relu, bias=sh[:], scale=a[:])
            nc.vector.tensor_tensor(hn[:], hn[:], maskb[:], Alu.mult)

            h2 = fullp.tile([P, span], f32, tag="h2")
            for lt in range(NTC):
                sl = slice(lt * P, (lt + 1) * P)
                ps = psA.tile([P, P], f32, tag="psA")
                nc.tensor.matmul(ps[:], w2t[:], hn[:, sl])
                nc.scalar.activation(h2[:, sl], ps[:], Act.Copy)
            s_stat = smallp.tile([P, 2], f32, tag="stat2")
            nc.vector.reduce_sum(s_stat[:, 0:1], h2[:],
                                 axis=mybir.AxisListType.X)
            sq = fullp.tile([P, span], f32, tag="sq")
            nc.vector.tensor_tensor(sq[:], h2[:], h2[:], Alu.mult)
            nc.vector.reduce_sum(s_stat[:, 1:2], sq[:],
                                 axis=mybir.AxisListType.X)
            nc.sync.dma_start(stat_out_d[:], s_stat[:])
            nc.sync.dma_start(h_out_d[:], h2[:])

    nc.compile()
    return nc


def _build_phase3():
    """NEFF3: out = transpose(relu(BN2(h2)))."""
    import concourse.bacc as bacc
    import concourse.mybir as mybir
    import concourse.tile as tile

    f32 = mybir.dt.float32
    Act = mybir.ActivationFunctionType
    span = NTC * P

    nc = bacc.Bacc("TRN2", target_bir_lowering=False, debug=False,
                   num_devices=NCORES)
    nc.sbuf_top = min(nc.sbuf_top, 192 * 1024)

    def din(name, shape):
        return nc.dram_tensor(name, shape, f32, kind="ExternalInput")

    h_d = din("h_in", [P, span])
    red_d = din("red", [P, 2])
    g2c_d = din("g2c", [P, 1]); b2c_d = din("b2c", [P, 1])
    ident_d = din("ident", [P, P])
    out_d = nc.dram_tensor("out", [span, D], f32, kind="ExternalOutput")

    with tile.TileContext(nc) as tc:
        with (
            tc.tile_pool(name="consts", bufs=1) as cp,
            tc.tile_pool(name="psA", bufs=2, space="PSUM") as psA,
            tc.tile_pool(name="full", bufs=1) as fullp,
            tc.tile_pool(name="small", bufs=1) as smallp,
        ):
            def load_const(name, dram, shape):
                t = cp.tile(shape, f32, tag=f"c_{name}")
                nc.sync.dma_start(t[:], dram[:])
                return t

            h2 = load_const("h", h_d, [P, span])
            red = load_const("red", red_d, [P, 2])
            g2c = load_const("g2c", g2c_d, [P, 1])
            b2c = load_const("b2c", b2c_d, [P, 1])
            ident = load_const("ident", ident_d, [P, P])

            a, sh = _bn_scale_shift(nc, mybir, smallp, red, g2c, b2c, 1)
            hn = fullp.tile([P, span], f32, tag="hn")
            nc.scalar.activation(hn[:], h2[:], Act.Relu, bias=sh[:], scale=a[:])

            stg = fullp.tile([P, span], f32, tag="stg")
            for lt in range(NTC):
                sl = slice(lt * P, (lt + 1) * P)
                ps = psA.tile([P, P], f32, tag="psA")
                nc.tensor.transpose(ps[:], hn[:, sl], ident[:])
                nc.scalar.activation(stg[:, sl], ps[:], Act.Copy)
            nc.sync.dma_start(
                out_d[:].rearrange("(g p) d -> p g d", p=P),
                stg[:].rearrange("p (g d) -> p g d", d=P))

    nc.compile()
    return nc


def kernel(**inputs):
    global last_results
    from concourse.bass_utils import run_bass_kernel_spmd

    in_maps, meta = _host_prep(inputs)
    if meta not in _cache:
        _cache[meta] = _build(meta)
    cores = list(range(NCORES))
    trace = bool(os.environ.get("KERNEL_TRACE"))

    if os.environ.get("KFUSE"):
        nc1 = _cache[meta]
        k1 = ("xt", "xbt", "wxt", "wbt", "w1t", "linbb", "iotab", "xct",
              "combw", "srcwp", "briwp", "dstloc", "w2t", "g1c", "b1c", "g2c", "b2c",
              "maskb", "ident")
        in1 = [{k: in_maps[c][k] for k in k1} for c in range(NCORES)]
        res1 = run_bass_kernel_spmd(nc1, in1, cores, trace=trace)
        last_results = (res1,)
        out = np.concatenate([res1.results[c]["out"]
                              for c in range(NCORES)], axis=0)
        return np.ascontiguousarray(out[:N])

    if "p2" not in _cache2:
        _cache2["p2"] = _build_phase2()
        _cache2["p3"] = _build_phase3()
    nc1, nc2, nc3 = _cache[meta], _cache2["p2"], _cache2["p3"]

    k1 = ("xt", "xbt", "wxt", "wbt", "w1t", "linbb", "iotab", "xct",
          "combw", "srcwp", "briwp", "dstloc")
    in1 = [{k: in_maps[c][k] for k in k1} for c in range(NCORES)]
    res1 = run_bass_kernel_spmd(nc1, in1, cores, trace=trace)
    red1 = np.sum([res1.results[c]["stat_out"] for c in range(NCORES)], axis=0)
    in2 = [{"h_in": res1.results[c]["h_out"], "red": red1,
            "w2t": in_maps[c]["w2t"], "g1c": in_maps[c]["g1c"],
            "b1c": in_maps[c]["b1c"], "maskb": in_maps[c]["maskb"]}
           for c in range(NCORES)]
    res2 = run_bass_kernel_spmd(nc2, in2, cores, trace=trace)
    red2 = np.sum([res2.results[c]["stat_out"] for c in range(NCORES)], axis=0)
    in3 = [{"h_in": res2.results[c]["h_out"], "red": red2,
            "g2c": in_maps[c]["g2c"], "b2c": in_maps[c]["b2c"],
            "ident": in_maps[c]["ident"]} for c in range(NCORES)]
    res3 = run_bass_kernel_spmd(nc3, in3, cores, trace=trace)

    last_results = (res1, res2, res3)
    out = np.concatenate([res3.results[c]["out"] for c in range(NCORES)], axis=0)
    return np.ascontiguousarray(out[:N])



# revision 44
# speedup vs baseline: 1.1673x; 1.0227x over previous
"""Trainium2 Bass kernel for nn_AdjacencyConv (GNN message passing).

Reference computation:
    msg  = relu(concat[x[src], x_bridge[bri]] @ lin_w.T + lin_b)   # [E, D]
    agg  = segment_sum(msg, dst, N)                                # [N, D]
    out  = agg + (1+eps)*x
    h    = relu(BN(out @ w1.T + b1)); h = relu(BN(h @ w2.T + b2))  # train-mode BN

Device algorithm (8-core SPMD, edges sharded by dst node-tile):
  Phase A (per core, replicated): build bf16 node tables in DRAM
        xW = x @ Wx.T            (Wx = lin_w[:, :D])      rows [0, N2)
        eW = x_bridge @ Wb.T + b (Wb = lin_w[:, D:])      rows [N2, N2+Nb2)
    so the per-edge linear factorizes: msg = relu(xW[src] + eW[bri]).
  Phase B: per dst node-tile of 128 nodes, one combined dma_gather per
    chunk fetches both rows of each edge from the concatenated table
    (edge-major bf16 [128e, D]), add + relu, then scatter-add via one-hot
    matmuls accumulating feature-major agg fp32 in PSUM. Residual and the
    first MLP linear run per-tile inside the loop. The first PRIME chunks
    instead use split x/e gathers: their x-gathers depend only on the xW
    half-table, so they start draining while eW still builds (hides most
    of Phase A). Phase B is bound by the SWDGE drain rate (~2.5 ns per
    descriptor across the 4 queues); bf16 halves gather bytes and gives
    one-pass PE matmuls vs fp32.
  Tail: 2 small follow-up NEFFs apply train-mode BN (biases b1/b2 cancel
    in BN and are dropped); the [128, 2] BN stat reduction across cores is
    host-stitched between NEFFs — a device AllReduce measured ~200us worse
    because it absorbs cross-core launch skew into the NEFF exec time.

Host side does only layout preprocessing: index sorting/padding/packing,
weight transposes, the 2KB BN stat reductions, and output assembly.
"""

import os
import numpy as np
import ml_dtypes

BF16 = ml_dtypes.bfloat16
N, NB, E, D = 10000, 20000, 640000, 128
P = 128
NCORES = 8
NTC = 10                    # node tiles per core
N2 = NCORES * NTC * P       # 10240 padded nodes
NB2 = 20096                 # x_bridge padded to multiple of 128
BN_EPS = 1e-5
GCHUNK = int(os.environ.get("KGCHUNK", "8"))  # batches per dma_gather
NQ = int(os.environ.get("KNQ", "4"))   # SWDGE queues
GBUFS = int(os.environ.get("KGBUFS", "10"))  # gather tile-pool depth
PRIME = int(os.environ.get("KPRIME", "20"))  # split-gather prefix chunks

_cache = {}

# exposed for test.py
last_results = None


def _chunks(B):
    """Chunk enumeration shared by host prep and kernel build:
    (lt, done, cb, batch_off, first_of_tile, last_of_tile)."""
    out = []
    gbase = 0
    for lt in range(NTC):
        nb = int(B[lt])
        done = 0
        while done < nb:
            cb = min(GCHUNK, nb - done)
            out.append((lt, done, cb, gbase + done,
                        done == 0, done + cb == nb))
            done += cb
        gbase += nb
    return out


def _pack_idx(idx16):
    """Wrap an int16 index vector for dma_gather: [16, n/16] replicated x8."""
    w = idx16.reshape(-1, 16).T
    return np.tile(w, (8, 1)).copy()


def _host_prep(inputs):
    x = np.asarray(inputs["x"], np.float32)
    xb = np.asarray(inputs["x_bridge"], np.float32)
    ei = np.asarray(inputs["edge_index"])
    bri = np.asarray(inputs["bridge_index"])
    lin_w = np.asarray(inputs["lin_w"], np.float32)
    lin_b = np.asarray(inputs["lin_b"], np.float32)
    eps = float(np.asarray(inputs["eps"]).reshape(-1)[0])
    w1 = np.asarray(inputs["w1"], np.float32)
    g1 = np.asarray(inputs["g1"], np.float32)
    beta1 = np.asarray(inputs["beta1"], np.float32)
    w2 = np.asarray(inputs["w2"], np.float32)
    g2 = np.asarray(inputs["g2"], np.float32)
    beta2 = np.asarray(inputs["beta2"], np.float32)

    src = ei[0].astype(np.int64)
    dst = ei[1].astype(np.int64)
    bri = bri.astype(np.int64)

    # sort edges by dst, bucket into 128-node tiles
    order = np.argsort(dst, kind="stable")
    dsts = dst[order]
    srcs = src[order]
    bris = bri[order]
    gt_bounds = np.searchsorted(dsts, np.arange(NCORES * NTC + 1) * P)

    # uniform program structure: per local tile lt, same batch count across cores
    cnt = np.diff(gt_bounds)  # edges per global tile (len 80)
    cnt = cnt.reshape(NCORES, NTC)
    ceils = -(-cnt // P)
    B = np.maximum(1, ceils.max(axis=0))  # batches per local tile (len NTC)
    SB = int(B.sum())          # total batches per core
    S = SB * P                 # padded edges per core

    src_pad = np.zeros((NCORES, S), np.int64)
    bri_pad = np.zeros((NCORES, S), np.int64)
    dloc_pad = np.full((NCORES, S), 999.0, np.float32)
    for c in range(NCORES):
        off = 0
        for lt in range(NTC):
            gt = c * NTC + lt
            a, b = gt_bounds[gt], gt_bounds[gt + 1]
            n = b - a
            src_pad[c, off:off + n] = srcs[a:b]
            bri_pad[c, off:off + n] = bris[a:b]
            dloc_pad[c, off:off + n] = (dsts[a:b] - gt * P).astype(np.float32)
            off += B[lt] * P
    # dstloc transposed: [128, SB], column j = batch j's 128 local-dst values
    dloc_T = np.ascontiguousarray(
        dloc_pad.reshape(NCORES, SB, P).transpose(0, 2, 1)).astype(BF16)

    # Gather index streams. The first `prime` chunks use split x/e gathers
    # (x-gathers only depend on the xW half-table, so they start while eW
    # still builds); the rest use one combined gather per chunk:
    # [src idx (cb*128), bri idx + N2 (cb*128)].
    chunks = _chunks(B)
    prime = min(PRIME, len(chunks))
    ep = sum(ch[2] for ch in chunks[:prime]) * P  # prefix edges
    comb = np.zeros((NCORES, 2 * (S - ep)), np.int64)
    for c in range(NCORES):
        woff = 0
        for (lt, done, cb, boff, _, _) in chunks[prime:]:
            off = boff * P
            ne = cb * P
            comb[c, woff:woff + ne] = src_pad[c, off:off + ne]
            comb[c, woff + ne:woff + 2 * ne] = bri_pad[c, off:off + ne] + N2
            woff += 2 * ne



    # feature-major padded inputs for the table build (bf16)
    xt = np.zeros((D, N2), BF16)
    xt[:, :N] = x.T.astype(BF16)
    xbt = np.zeros((D, NB2), BF16)
    xbt[:, :NB] = xb.T.astype(BF16)

    wxt = np.ascontiguousarray(lin_w[:, :D].T).astype(BF16)   # [in_f, out]
    wbt = np.ascontiguousarray(lin_w[:, D:].T).astype(BF16)
    w1t = np.ascontiguousarray(w1.T)
    w2t = np.ascontiguousarray(w2.T)
    linbb = np.tile(lin_b[None, :], (P, 1)).astype(np.float32)   # [128, 128]
    iotab = np.tile(np.arange(P, dtype=np.float32)[None, :], (P, GCHUNK)).astype(BF16)
    ident = np.eye(P, dtype=np.float32)

    g1c = np.ascontiguousarray(g1[:, None])
    b1c = np.ascontiguousarray(beta1[:, None])
    g2c = np.ascontiguousarray(g2[:, None])
    b2c = np.ascontiguousarray(beta2[:, None])

    # per-core residual slice (feature-major) and validity mask
    span = NTC * P
    xct = np.zeros((NCORES, D, span), np.float32)
    maskb = np.zeros((NCORES, P, span), np.float32)
    for c in range(NCORES):
        c0 = c * span
        v = min(max(N - c0, 0), span)
        if v > 0:
            xct[c, :, :v] = x.T[:, c0:c0 + v]
            maskb[c, :, :v] = 1.0

    in_maps = []
    for c in range(NCORES):
        in_maps.append({
            "xt": xt, "xbt": xbt,
            "wxt": wxt, "wbt": wbt, "w1t": w1t, "w2t": w2t,
            "linbb": linbb, "iotab": iotab, "ident": ident,
            "g1c": g1c, "b1c": b1c, "g2c": g2c, "b2c": b2c,
            "xct": np.ascontiguousarray(xct[c]),
            "maskb": np.ascontiguousarray(maskb[c]),
            "combw": _pack_idx(comb[c].astype(np.int16)),
            "srcwp": _pack_idx(src_pad[c, :ep].astype(np.int16)),
            "briwp": _pack_idx(bri_pad[c, :ep].astype(np.int16)),
            "dstloc": np.ascontiguousarray(dloc_T[c]),
        })
    meta = (tuple(int(b) for b in B), 1.0 + eps)
    return in_maps, meta


def _build(meta):
    import concourse.bacc as bacc
    import concourse.mybir as mybir
    import concourse.tile as tile

    KP = 3  # full fused kernel
    B, resid_scale = meta
    SB = sum(B)
    S = SB * P
    f32 = mybir.dt.float32
    bf16 = mybir.dt.bfloat16
    i16 = mybir.dt.int16
    Alu = mybir.AluOpType
    Act = mybir.ActivationFunctionType
    span = NTC * P

    NQr = int(os.environ.get("KNQ", "4"))
    scratch = int(os.environ.get("KSCRATCH", "16384"))
    nc = bacc.Bacc("TRN2", target_bir_lowering=False, debug=False,
                   num_devices=NCORES, num_swdge_queues=NQr,
                   dynamic_dma_scratch_size=scratch)
    # Leave the top of SBUF for the runtime's SWDGE descriptor rings /
    # DynamicDMAScratch carveout — allocating into it wedges the device.
    nc.sbuf_top = min(nc.sbuf_top, 192 * 1024)

    def din(name, shape, dt=f32):
        return nc.dram_tensor(name, shape, dt, kind="ExternalInput")

    fuse = bool(os.environ.get("KFUSE"))
    xt_d = din("xt", [D, N2], bf16); xbt_d = din("xbt", [D, NB2], bf16)
    wxt_d = din("wxt", [D, D], bf16); wbt_d = din("wbt", [D, D], bf16)
    w1t_d = din("w1t", [D, D])
    linbb_d = din("linbb", [P, D]); iotab_d = din("iotab", [P, GCHUNK * P], bf16)
    xct_d = din("xct", [D, span])
    chunks = _chunks(B)
    prime = min(PRIME, len(chunks))
    EP = sum(ch[2] for ch in chunks[:prime]) * P
    combw_d = din("combw", [128, 2 * (S - EP) // 16], i16)
    srcwp_d = din("srcwp", [128, EP // 16], i16)
    briwp_d = din("briwp", [128, EP // 16], i16)
    dstloc_d = din("dstloc", [P, SB], bf16)
    if fuse:
        w2t_d = din("w2t", [D, D])
        g1c_d = din("g1c", [P, 1]); b1c_d = din("b1c", [P, 1])
        g2c_d = din("g2c", [P, 1]); b2c_d = din("b2c", [P, 1])
        maskb_d = din("maskb", [P, span]); ident_d = din("ident", [P, P])
        out_d = nc.dram_tensor("out", [span, D], f32, kind="ExternalOutput")
        bn_in = {i: nc.dram_tensor(f"bn_in{i}", [P, 2], f32) for i in (1, 2)}
        bn_out = {i: nc.dram_tensor(f"bn_out{i}", [P, 2], f32) for i in (1, 2)}
    else:
        h_out_d = nc.dram_tensor("h_out", [P, span], f32, kind="ExternalOutput")
        stat_out_d = nc.dram_tensor("stat_out", [P, 2], f32,
                                    kind="ExternalOutput")

    ct_t = nc.dram_tensor("ct_tab", [N2 + NB2, D], bf16)
    with tile.TileContext(nc) as tc:
        with (
            tc.tile_pool(name="consts", bufs=1) as cp,
            tc.tile_pool(name="pa_src", bufs=2) as pa_src,
            tc.tile_pool(name="pa_stg", bufs=2) as pa_stg,
            tc.tile_pool(name="psA", bufs=2, space="PSUM") as psA,
            tc.tile_pool(name="psB", bufs=2, space="PSUM") as psB,
            tc.tile_pool(name="gx", bufs=GBUFS) as gxp,
            tc.tile_pool(name="px", bufs=max(1, min(PRIME, 20))) as pxp,
            tc.tile_pool(name="pe", bufs=4) as pep,
            tc.tile_pool(name="oh", bufs=GBUFS) as ohp,
            tc.tile_pool(name="full", bufs=1) as fullp,
            tc.tile_pool(name="small", bufs=1) as smallp,
        ):
            def load_const(name, dram, shape, dt=f32, eng=None):
                t = cp.tile(shape, dt, tag=f"c_{name}")
                (eng or nc.sync).dma_start(t[:], dram[:])
                return t

            wxt = load_const("wxt", wxt_d, [D, D], bf16)
            wbt = load_const("wbt", wbt_d, [D, D], bf16)
            w1t = load_const("w1t", w1t_d, [D, D], eng=nc.scalar)
            linbb = load_const("linbb", linbb_d, [P, D])
            iotab = load_const("iotab", iotab_d, [P, GCHUNK * P], bf16)
            xct = load_const("xct", xct_d, [D, span], eng=nc.scalar)
            combw = load_const("combw", combw_d,
                               [128, 2 * (S - EP) // 16], i16, eng=nc.scalar)
            srcwp = load_const("srcwp", srcwp_d, [128, EP // 16], i16,
                               eng=nc.scalar)
            briwp = load_const("briwp", briwp_d, [128, EP // 16], i16,
                               eng=nc.scalar)
            dstloc = load_const("dstloc", dstloc_d, [P, SB], bf16, eng=nc.scalar)
            if fuse:
                w2t = load_const("w2t", w2t_d, [D, D])
                g1c = load_const("g1c", g1c_d, [P, 1])
                b1c = load_const("b1c", b1c_d, [P, 1])
                g2c = load_const("g2c", g2c_d, [P, 1])
                b2c = load_const("b2c", b2c_d, [P, 1])
                maskb = load_const("maskb", maskb_d, [P, span])
                ident = load_const("ident", ident_d, [P, P])

            # ---------------- Phase A: node tables in DRAM ----------------
            CW = 2048  # source columns per chunk

            def build_table(src_dram, ncols, w_sbuf, row_off, add_bias):
                for c0 in range(0, ncols, CW):
                    w = min(CW, ncols - c0)
                    s = pa_src.tile([D, CW], bf16, tag="pa_src")
                    nc.sync.dma_start(s[:, :w], src_dram[:, c0:c0 + w])
                    g = pa_stg.tile([P, CW], bf16, tag="pa_stg")
                    for q0 in range(0, w, 512):
                        qw = min(512, w - q0)
                        ps = psA.tile([P, 512], f32, tag="psA")
                        for j in range(qw // P):
                            nc.tensor.matmul(
                                ps[:, j * P:(j + 1) * P],
                                s[:, q0 + j * P:q0 + (j + 1) * P], w_sbuf[:])
                        sl = g[:, q0:q0 + qw]
                        if add_bias:
                            lb = linbb[:].rearrange(
                                "p (b d) -> p b d", b=1).to_broadcast(
                                (P, qw // P, P))
                            nc.vector.tensor_tensor(
                                sl.rearrange("p (b c) -> p b c", c=P),
                                ps[:, :qw].rearrange("p (b c) -> p b c", c=P),
                                lb, Alu.add)
                        else:
                            nc.scalar.activation(sl, ps[:, :qw], Act.Copy)
                    r0 = row_off + c0
                    nc.scalar.dma_start(
                        ct_t[r0:r0 + w, :].rearrange("(g p) d -> p g d", p=P),
                        g[:, :w].rearrange("p (g d) -> p g d", d=P))

            build_table(xt_d, N2, wxt, 0, False)
            build_table(xbt_d, NB2, wbt, N2, True)

            # ---------------- Phase B: gather + scatter-add ----------------
            nchunks = len(chunks)
            outT = fullp.tile([P, span], f32, tag="outT")
            h1 = fullp.tile([P, span], f32, tag="h")
            if not fuse:
                pstat = smallp.tile([P, NTC], f32, tag="pstat")
                pstat2 = smallp.tile([P, NTC], f32, tag="pstat2")
            gq = [0]

            # Primed prefix: x-gathers issued back-to-back; they only
            # depend on the xW half-table, so they drain while eW builds.
            pxs = []
            for k in range(prime):
                lt, done, cb, boff, _, _ = chunks[k]
                ne = cb * P
                goff = boff * (P // 16)
                px = pxp.tile([P, GCHUNK, D], bf16, tag="px")
                nc.gpsimd.dma_gather(
                    px[:, :cb, :], ct_t[0:N2, :],
                    srcwp[:, goff:goff + ne // 16],
                    ne, ne, D, single_packet=False, queue_num=gq[0] % NQr)
                gq[0] += 1
                pxs.append(px)

            aggT = None
            csoff = 0  # combined-stream batch offset for suffix chunks
            for k in range(nchunks):
                lt, done, cb, boff, first_of_tile, last_of_tile = chunks[k]
                ne = cb * P
                if k < prime:
                    pe = pep.tile([P, GCHUNK, D], bf16, tag="pe")
                    goff = boff * (P // 16)
                    nc.gpsimd.dma_gather(
                        pe[:, :cb, :], ct_t[N2:N2 + NB2, :],
                        briwp[:, goff:goff + ne // 16],
                        ne, ne, D, single_packet=False, queue_num=gq[0] % NQr)
                    gq[0] += 1
                    gx = pxs[k]
                    if first_of_tile:
                        aggT = psB.tile([P, P], f32, tag="aggT")
                    nc.vector.tensor_tensor(gx[:, :cb, :], gx[:, :cb, :],
                                            pe[:, :cb, :], Alu.add)
                else:
                    goff = 2 * csoff * (P // 16)
                    csoff += cb
                    gx = gxp.tile([P, 2 * GCHUNK, D], bf16, tag="gx")
                    nc.gpsimd.dma_gather(
                        gx[:, :2 * cb, :], ct_t[:],
                        combw[:, goff:goff + 2 * ne // 16],
                        2 * ne, 2 * ne, D, single_packet=False,
                        queue_num=gq[0] % NQr)
                    gq[0] += 1
                    if first_of_tile:
                        aggT = psB.tile([P, P], f32, tag="aggT")
                    nc.vector.tensor_tensor(gx[:, :cb, :], gx[:, :cb, :],
                                            gx[:, cb:2 * cb, :], Alu.add)
                nc.scalar.activation(gx[:, :cb, :], gx[:, :cb, :], Act.Relu)

                oh = ohp.tile([P, GCHUNK * P], bf16, tag="oh")
                dl = dstloc[:, boff:boff + cb]
                dl_rep = dl.to_broadcast((P, cb, P))
                nc.vector.tensor_tensor(
                    oh[:, :cb * P].rearrange("p (b c) -> p b c", c=P),
                    iotab[:, :cb * P].rearrange("p (b c) -> p b c", c=P),
                    dl_rep, Alu.is_equal)
                for b in range(cb):
                    nc.tensor.matmul(
                        aggT[:], gx[:, b, :], oh[:, b * P:(b + 1) * P],
                        start=(first_of_tile and b == 0),
                        stop=(last_of_tile and b == cb - 1))

                if last_of_tile:
                    # residual + first MLP linear for this finished tile
                    sl = slice(lt * P, (lt + 1) * P)
                    nc.vector.scalar_tensor_tensor(
                        outT[:, sl], xct[:, sl], float(resid_scale), aggT[:],
                        Alu.mult, Alu.add)
                    ps = psA.tile([P, P], f32, tag="psA")
                    nc.tensor.matmul(ps[:], w1t[:], outT[:, sl])
                    nc.scalar.activation(h1[:, sl], ps[:], Act.Copy)
                    if not fuse:
                        # per-tile BN1 partial stats + h1 writeback, hidden
                        # under the gather-bound loop
                        sqt = ohp.tile([P, P], f32, tag="sqt")
                        nc.vector.tensor_tensor(sqt[:], h1[:, sl], h1[:, sl],
                                                Alu.mult)
                        nc.vector.reduce_sum(pstat[:, lt:lt + 1], h1[:, sl],
                                             axis=mybir.AxisListType.X)
                        nc.vector.reduce_sum(pstat2[:, lt:lt + 1], sqt[:],
                                             axis=mybir.AxisListType.X)
                        nc.sync.dma_start(h_out_d[:, sl], h1[:, sl])

            # ---------------- Phase C: MLP + BN (feature-major) ----------------
            def bn_layer(idx, h_full, gcol, bcol, apply_mask, skip_coll=False):
                s_stat = smallp.tile([P, 2], f32, tag=f"stat{idx}")
                nc.vector.reduce_sum(s_stat[:, 0:1], h_full[:],
                                     axis=mybir.AxisListType.X)
                sq = fullp.tile([P, span], f32, tag="sq")
                nc.vector.tensor_tensor(sq[:], h_full[:], h_full[:], Alu.mult)
                nc.vector.reduce_sum(s_stat[:, 1:2], sq[:],
                                     axis=mybir.AxisListType.X)
                nc.sync.dma_start(bn_in[idx][:], s_stat[:])
                if not skip_coll:
                    nc.gpsimd.collective_compute(
                        "AllReduce", Alu.add,
                        replica_groups=[list(range(NCORES))],
                        ins=[bn_in[idx][:]], outs=[bn_out[idx][:]])
                red = smallp.tile([P, 2], f32, tag=f"red{idx}")
                nc.sync.dma_start(red[:], bn_in[idx][:] if skip_coll else bn_out[idx][:])
                mu = smallp.tile([P, 1], f32, tag=f"mu{idx}")
                nc.vector.tensor_scalar(mu[:], red[:, 0:1], 1.0 / N, None, Alu.mult)
                var = smallp.tile([P, 1], f32, tag=f"var{idx}")
                mu2 = smallp.tile([P, 1], f32, tag=f"mu2{idx}")
                nc.vector.tensor_tensor(mu2[:], mu[:], mu[:], Alu.mult)
                e2 = smallp.tile([P, 1], f32, tag=f"e2{idx}")
                nc.vector.tensor_scalar(e2[:], red[:, 1:2], 1.0 / N, None, Alu.mult)
                nc.vector.tensor_tensor(var[:], e2[:], mu2[:], Alu.subtract)
                vep = smallp.tile([P, 1], f32, tag=f"vep{idx}")
                nc.vector.tensor_scalar(vep[:], var[:], BN_EPS, None, Alu.add)
                std = smallp.tile([P, 1], f32, tag=f"std{idx}")
                nc.scalar.activation(std[:], vep[:], Act.Sqrt)
                rstd = smallp.tile([P, 1], f32, tag=f"rstd{idx}")
                nc.vector.reciprocal(rstd[:], std[:])
                a = smallp.tile([P, 1], f32, tag=f"a{idx}")
                nc.vector.tensor_tensor(a[:], gcol[:], rstd[:], Alu.mult)
                sh = smallp.tile([P, 1], f32, tag=f"sh{idx}")
                tmp = smallp.tile([P, 1], f32, tag=f"tmp{idx}")
                nc.vector.tensor_tensor(tmp[:], mu[:], a[:], Alu.mult)
                nc.vector.tensor_tensor(sh[:], bcol[:], tmp[:], Alu.subtract)
                hn = fullp.tile([P, span], f32, tag="hn")
                nc.scalar.activation(hn[:], h_full[:], Act.Relu,
                                     bias=sh[:], scale=a[:])
                if apply_mask:
                    nc.vector.tensor_tensor(hn[:], hn[:], maskb[:], Alu.mult)
                return hn

            def linear(w_sbuf, src_full, idx):
                h = fullp.tile([P, span], f32, tag="h")
                for lt in range(NTC):
                    sl = slice(lt * P, (lt + 1) * P)
                    ps = psA.tile([P, P], f32, tag="psA")
                    nc.tensor.matmul(ps[:], w_sbuf[:], src_full[:, sl])
                    nc.scalar.activation(h[:, sl], ps[:], Act.Copy)
                return h

            if fuse:
                hn1 = bn_layer(1, h1, g1c, b1c, apply_mask=True)
                h2 = linear(w2t, hn1, 2)
                hn2 = bn_layer(2, h2, g2c, b2c, apply_mask=False)
                stg = fullp.tile([P, span], f32, tag="stg")
                for lt in range(NTC):
                    sl = slice(lt * P, (lt + 1) * P)
                    ps = psA.tile([P, P], f32, tag="psA")
                    nc.tensor.transpose(ps[:], hn2[:, sl], ident[:])
                    nc.scalar.activation(stg[:, sl], ps[:], Act.Copy)
                nc.sync.dma_start(
                    out_d[:].rearrange("(g p) d -> p g d", p=P),
                    stg[:].rearrange("p (g d) -> p g d", d=P))
            else:
                # fold the per-tile partials; the 8-way BN stat reduction
                # happens on the host between NEFFs (h1 already written)
                s_stat = smallp.tile([P, 2], f32, tag="stat1")
                nc.vector.reduce_sum(s_stat[:, 0:1], pstat[:],
                                     axis=mybir.AxisListType.X)
                nc.vector.reduce_sum(s_stat[:, 1:2], pstat2[:],
                                     axis=mybir.AxisListType.X)
                nc.sync.dma_start(stat_out_d[:], s_stat[:])

    nc.compile()
    return nc




_cache2 = {}


def _bn_scale_shift(nc, mybir, smallp, red, gcol, bcol, idx):
    """Device-side BN coefficients from reduced stats: a = g*rstd, sh = b - mu*a."""
    f32 = mybir.dt.float32
    Alu = mybir.AluOpType
    Act = mybir.ActivationFunctionType
    mu = smallp.tile([P, 1], f32, tag=f"mu{idx}")
    nc.vector.tensor_scalar(mu[:], red[:, 0:1], 1.0 / N, None, Alu.mult)
    mu2 = smallp.tile([P, 1], f32, tag=f"mu2{idx}")
    nc.vector.tensor_tensor(mu2[:], mu[:], mu[:], Alu.mult)
    e2 = smallp.tile([P, 1], f32, tag=f"e2{idx}")
    nc.vector.tensor_scalar(e2[:], red[:, 1:2], 1.0 / N, None, Alu.mult)
    var = smallp.tile([P, 1], f32, tag=f"var{idx}")
    nc.vector.tensor_tensor(var[:], e2[:], mu2[:], Alu.subtract)
    vep = smallp.tile([P, 1], f32, tag=f"vep{idx}")
    nc.vector.tensor_scalar(vep[:], var[:], BN_EPS, None, Alu.add)
    std = smallp.tile([P, 1], f32, tag=f"std{idx}")
    nc.scalar.activation(std[:], vep[:], Act.Sqrt)
    rstd = smallp.tile([P, 1], f32, tag=f"rstd{idx}")
    nc.vector.reciprocal(rstd[:], std[:])
    a = smallp.tile([P, 1], f32, tag=f"a{idx}")
    nc.vector.tensor_tensor(a[:], gcol[:], rstd[:], Alu.mult)
    tmp = smallp.tile([P, 1], f32, tag=f"tmp{idx}")
    nc.vector.tensor_tensor(tmp[:], mu[:], a[:], Alu.mult)
    sh = smallp.tile([P, 1], f32, tag=f"sh{idx}")
    nc.vector.tensor_tensor(sh[:], bcol[:], tmp[:], Alu.subtract)
    return a, sh


def _build_phase2():
    """NEFF2: h1n = mask*relu(BN1(h1)); h2 = h1n @ w2.T; per-core stats of h2."""
    import concourse.bacc as bacc
    import concourse.mybir as mybir
    import concourse.tile as tile

    f32 = mybir.dt.float32
    Alu = mybir.AluOpType
    Act = mybir.ActivationFunctionType
    span = NTC * P

    nc = bacc.Bacc("TRN2", target_bir_lowering=False, debug=False,
                   num_devices=NCORES)
    nc.sbuf_top = min(nc.sbuf_top, 192 * 1024)

    def din(name, shape):
        return nc.dram_tensor(name, shape, f32, kind="ExternalInput")

    h_d = din("h_in", [P, span])
    red_d = din("red", [P, 2])
    w2t_d = din("w2t", [D, D])
    g1c_d = din("g1c", [P, 1]); b1c_d = din("b1c", [P, 1])
    maskb_d = din("maskb", [P, span])
    h_out_d = nc.dram_tensor("h_out", [P, span], f32, kind="ExternalOutput")
    stat_out_d = nc.dram_tensor("stat_out", [P, 2], f32, kind="ExternalOutput")

    with tile.TileContext(nc) as tc:
        with (
            tc.tile_pool(name="consts", bufs=1) as cp,
            tc.tile_pool(name="psA", bufs=2, space="PSUM") as psA,
            tc.tile_pool(name="full", bufs=1) as fullp,
            tc.tile_pool(name="small", bufs=1) as smallp,
        ):
            def load_const(name, dram, shape):
                t = cp.tile(shape, f32, tag=f"c_{name}")
                nc.sync.dma_start(t[:], dram[:])
                return t

            h1 = load_const("h", h_d, [P, span])
            red = load_const("red", red_d, [P, 2])
            w2t = load_const("w2t", w2t_d, [D, D])
            g1c = load_const("g1c", g1c_d, [P, 1])
            b1c = load_const("b1c", b1c_d, [P, 1])
            maskb = load_const("maskb", maskb_d, [P, span])

            a, sh = _bn_scale_shift(nc, mybir, smallp, red, g1c, b1c, 0)
            hn = fullp.tile([P, span], f32, tag="hn")
            nc.scalar.activation(hn[:], h1[:], Act.Relu, bias=sh[:], scale=a[:])
            nc.vector.tensor_tensor(hn[:], hn[:], maskb[:], Alu.mult)

            h2 = fullp.tile([P, span], f32, tag="h2")
            for lt in range(NTC):
                sl = slice(lt * P, (lt + 1) * P)
                ps = psA.tile([P, P], f32, tag="psA")
                nc.tensor.matmul(ps[:], w2t[:], hn[:, sl])
                nc.scalar.activation(h2[:, sl], ps[:], Act.Copy)
            s_stat = smallp.tile([P, 2], f32, tag="stat2")
            nc.vector.reduce_sum(s_stat[:, 0:1], h2[:],
                                 axis=mybir.AxisListType.X)
            sq = fullp.tile([P, span], f32, tag="sq")
            nc.vector.tensor_tensor(sq[:], h2[:], h2[:], Alu.mult)
            nc.vector.reduce_sum(s_stat[:, 1:2], sq[:],
                                 axis=mybir.AxisListType.X)
            nc.sync.dma_start(stat_out_d[:], s_stat[:])
            nc.sync.dma_start(h_out_d[:], h2[:])

    nc.compile()
    return nc


def _build_phase3():
    """NEFF3: out = transpose(relu(BN2(h2)))."""
    import concourse.bacc as bacc
    import concourse.mybir as mybir
    import concourse.tile as tile

    f32 = mybir.dt.float32
    Act = mybir.ActivationFunctionType
    span = NTC * P

    nc = bacc.Bacc("TRN2", target_bir_lowering=False, debug=False,
                   num_devices=NCORES)
    nc.sbuf_top = min(nc.sbuf_top, 192 * 1024)

    def din(name, shape):
        return nc.dram_tensor(name, shape, f32, kind="ExternalInput")

    h_d = din("h_in", [P, span])
    red_d = din("red", [P, 2])
    g2c_d = din("g2c", [P, 1]); b2c_d = din("b2c", [P, 1])
    ident_d = din("ident", [P, P])
    out_d = nc.dram_tensor("out", [span, D], f32, kind="ExternalOutput")

    with tile.TileContext(nc) as tc:
        with (
            tc.tile_pool(name="consts", bufs=1) as cp,
            tc.tile_pool(name="psA", bufs=2, space="PSUM") as psA,
            tc.tile_pool(name="full", bufs=1) as fullp,
            tc.tile_pool(name="small", bufs=1) as smallp,
        ):
            def load_const(name, dram, shape):
                t = cp.tile(shape, f32, tag=f"c_{name}")
                nc.sync.dma_start(t[:], dram[:])
                return t

            h2 = load_const("h", h_d, [P, span])
            red = load_const("red", red_d, [P, 2])
            g2c = load_const("g2c", g2c_d, [P, 1])
            b2c = load_const("b2c", b2c_d, [P, 1])
            ident = load_const("ident", ident_d, [P, P])

            a, sh = _bn_scale_shift(nc, mybir, smallp, red, g2c, b2c, 1)
            hn = fullp.tile([P, span], f32, tag="hn")
            nc.scalar.activation(hn[:], h2[:], Act.Relu, bias=sh[:], scale=a[:])

            stg = fullp.tile([P, span], f32, tag="stg")
            for lt in range(NTC):
                sl = slice(lt * P, (lt + 1) * P)
                ps = psA.tile([P, P], f32, tag="psA")
                nc.tensor.transpose(ps[:], hn[:, sl], ident[:])
                nc.scalar.activation(stg[:, sl], ps[:], Act.Copy)
            nc.sync.dma_start(
                out_d[:].rearrange("(g p) d -> p g d", p=P),
                stg[:].rearrange("p (g d) -> p g d", d=P))

    nc.compile()
    return nc


def kernel(**inputs):
    global last_results
    from concourse.bass_utils import run_bass_kernel_spmd

    in_maps, meta = _host_prep(inputs)
    if meta not in _cache:
        _cache[meta] = _build(meta)
    cores = list(range(NCORES))
    trace = bool(os.environ.get("KERNEL_TRACE"))

    if os.environ.get("KFUSE"):
        nc1 = _cache[meta]
        k1 = ("xt", "xbt", "wxt", "wbt", "w1t", "linbb", "iotab", "xct",
              "combw", "srcwp", "briwp", "dstloc", "w2t", "g1c", "b1c", "g2c", "b2c",
              "maskb", "ident")
        in1 = [{k: in_maps[c][k] for k in k1} for c in range(NCORES)]
        res1 = run_bass_kernel_spmd(nc1, in1, cores, trace=trace)
        last_results = (res1,)
        out = np.concatenate([res1.results[c]["out"]
                              for c in range(NCORES)], axis=0)
        return np.ascontiguousarray(out[:N])

    if "p2" not in _cache2:
        _cache2["p2"] = _build_phase2()
        _cache2["p3"] = _build_phase3()
    nc1, nc2, nc3 = _cache[meta], _cache2["p2"], _cache2["p3"]

    k1 = ("xt", "xbt", "wxt", "wbt", "w1t", "linbb", "iotab", "xct",
          "combw", "srcwp", "briwp", "dstloc")
    in1 = [{k: in_maps[c][k] for k in k1} for c in range(NCORES)]
    res1 = run_bass_kernel_spmd(nc1, in1, cores, trace=trace)
    red1 = np.sum([res1.results[c]["stat_out"] for c in range(NCORES)], axis=0)
    in2 = [{"h_in": res1.results[c]["h_out"], "red": red1,
            "w2t": in_maps[c]["w2t"], "g1c": in_maps[c]["g1c"],
            "b1c": in_maps[c]["b1c"], "maskb": in_maps[c]["maskb"]}
           for c in range(NCORES)]
    res2 = run_bass_kernel_spmd(nc2, in2, cores, trace=trace)
    red2 = np.sum([res2.results[c]["stat_out"] for c in range(NCORES)], axis=0)
    in3 = [{"h_in": res2.results[c]["h_out"], "red": red2,
            "g2c": in_maps[c]["g2c"], "b2c": in_maps[c]["b2c"],
            "ident": in_maps[c]["ident"]} for c in range(NCORES)]
    res3 = run_bass_kernel_spmd(nc3, in3, cores, trace=trace)

    last_results = (res1, res2, res3)
    out = np.concatenate([res3.results[c]["out"] for c in range(NCORES)], axis=0)
    return np.ascontiguousarray(out[:N])

